# revision 26
# baseline (speedup 1.0000x reference)
"""Trainium2 Bass kernel for AttentionWithFP4Projections.

Sharding: tensor-parallel over heads across 8 cores (4 heads each, both
batches). Each core computes q/k/v for its 256 output dims, full causal
attention for its heads, and a partial o_proj (its 256-dim slice of the
contraction); partials are summed on the host (no device collectives).

Numerics: FP4 fake-quant reproduced bit-exactly (magic-constant rounding
within fp32-ALU-exact ranges), split DVE (mults/select) + GpSimd (bit ops).
Matmul dtypes chosen for PE speed (1 cyc/row instead of fp32's 4):
  - q/k/v projections + scores: fp16 operands (quantized values rounded
    to fp16; ~5e-4 rel, fine through softmax),
  - PV (o accumulation): float32r (fp22) - o feeds the o-quant whose
    bucket decisions amplify pre-quant error, so keep it >= fp22,
  - o_proj: bf16 (post-quant, no amplification).
Softmax without max-subtraction (max scaled score ~5, no overflow);
normalization folded into the o-quant scale via a ones-column in V.
"""
import sys
import types
from contextlib import ExitStack

import numpy as np

# The NTFF profiling hook module is missing in this image; shim it so
# run_bass_kernel_spmd(trace=True) works (used by test.py, harmless here).
if 'antenv.axon_hooks' not in sys.modules:
    _m = types.ModuleType('antenv.axon_hooks')
    _m._hook = None
    _m.set_axon_ntff_profile_hook = lambda h: setattr(_m, '_hook', h)
    _m.get_axon_ntff_profile_hook = lambda: _m._hook
    sys.modules['antenv.axon_hooks'] = _m
    try:
        from trn_agent_boot.trn_boot import _ntff_profile_via_ctypes
        _m._hook = _ntff_profile_via_ctypes('/opt/axon/libaxon_pjrt.so')
    except Exception:
        pass

import concourse.mybir as mybir
import concourse.tile as tile
from concourse import bacc
from concourse import bass_utils
from concourse.masks import make_identity

# ---- custom fused DVE ops for the FP4 grid rounding ----------------------
# Extends the module-level registry (the documented add-an-op flow; the
# dve_ops.py file itself is read-only in this image).  Rows 17/18 are free
# (16 stock ops occupy 1..16; the byte-36 row field allows [1, 0x20)).
import concourse.dve_ops as _dvo
from concourse.dve_spec import (Spec as _Spec, Src0 as _Src0, Src1 as _Src1,
                                C0 as _C0, C1 as _C1, C2 as _C2,
                                Zero as _Zero, One as _One,
                                maxx as _maxx, minn as _minn, sq as _sq)


def _register_fp4_ops():
    import numpy as _np
    by_name = {o.name: o for o in _dvo.OPS}
    if "FP4_HI_ANT" in by_name:
        return by_name["FP4_HI_ANT"], by_name["FP4_LO_MERGE_ANT"]
    _ysq = _sq(_Src0)
    _i1 = _C0 < _ysq   # s0 = 2.5**2
    _i2 = _C1 < _ysq   # s1 = 3.5**2
    _i3 = _C2 < _ysq   # imm2 = 5**2
    # habs = 3*(|y|>2.5) + (|y|>3.5) + 2*(|y|>5), via (i1+i3)*2 + i2 + i1
    specB = _Spec(
        body=(_i1 + _i3) * (_One + _One) + _i2 + _i1,
        reference=lambda in0, s0, s1, imm2: (
            ((in0 * in0 > s0).astype(_np.float32) + (in0 * in0 > imm2)) * 2.0
            + (in0 * in0 > s1) + (in0 * in0 > s0)),
    )
    # g = clamp(magic_round(y), -2, 2) * (y*y <= 6.25) + in1(signed high)
    specA = _Spec(
        body=_minn(_maxx((_Src0 + _C0) - _C0, _C2), _Zero - _C2)
        * (_sq(_Src0) <= _C1) + _Src1,
        reference=lambda in0, in1, s0, s1, imm2: _np.clip(
            (in0 + _np.float32(s0)).astype(_np.float32) - _np.float32(s0),
            imm2, -imm2) * (in0 * in0 <= s1) + in1,
    )
    opB = _dvo.DveOp("FP4_HI_ANT", specB, subdim=False,
                     uops_sha={"v3": "176720cc7ee0a7f8",
                               "v4": "014accfcba4ba70e"})
    opA = _dvo.DveOp("FP4_LO_MERGE_ANT", specA, subdim=False,
                     uops_sha={"v3": "b57d557c01bd412c",
                               "v4": "780a6585d0fe9dbb"})
    _dvo.OPS.extend([opB, opA])
    _dvo._SUB_OPCODE_FOR_NAME[opB.name] = 17
    _dvo._SUB_OPCODE_FOR_NAME[opA.name] = 18
    _dvo.CUSTOM_DVE_SPECS[opB.name] = specB
    _dvo.CUSTOM_DVE_SPECS[opA.name] = specA
    return opB, opA


FP4_HI, FP4_LO_MERGE = _register_fp4_ops()

F32 = mybir.dt.float32
F32R = mybir.dt.float32r
F16 = mybir.dt.float16
BF16 = mybir.dt.bfloat16
I32 = mybir.dt.int32
ALU = mybir.AluOpType
ACTF = mybir.ActivationFunctionType

NCORES = 8
B, S, HID = 2, 2048, 2048
T = B * S                     # 4096 tokens
NH, HD = 32, 64               # heads, head dim
HPC = NH // NCORES            # 4 heads per core
OD = HPC * HD                 # 256 output dims per core
SPC = S // NCORES             # 256 tokens per batch per core
TC = 512                      # token-chunk width for projections
QW = 512                      # quantization sub-width (temp buffer size)
MAGIC = 6291456.0             # 1.5*2^22: +/- rounds fp32 to multiples of 0.5
NEG = -1.0e30


def _quant(nc, sb_tmp, out_ap, in_ap, scale_ap, rs6_ap, W, P=128):
    """FP4 fake-quant of in_ap [P, W] -> out_ap, given per-block scale and
    rs6 (=6/amax) [P, W//16].  Matches the jnp reference up to 1-ulp
    boundary/tie cases (reciprocal-based scale path, squared-bound
    compares).  Uses two fused custom DVE ops: 6 big passes total."""
    nb = W // 16
    y = sb_tmp.tile([128, QW], F32, tag="qt_y", name="qt_y")[:P, :W]
    nc.vector.tensor_tensor(
        out=y.rearrange("p (b s) -> p b s", s=16),
        in0=in_ap.rearrange("p (b s) -> p b s", s=16),
        in1=rs6_ap.unsqueeze(2).broadcast_to([P, nb, 16]),
        op=ALU.mult)
    h = sb_tmp.tile([128, QW], F32, tag="qt_h", name="qt_h")[:P, :W]
    nc.vector._custom_dve(FP4_HI, out=h, in0=y,
                          s0=6.25, s1=12.25, imm2=25.0)
    sbit = sb_tmp.tile([128, QW], I32, tag="qt_s", name="qt_s")[:P, :W]
    nc.vector.tensor_scalar(out=sbit, in0=in_ap.bitcast(I32),
                            scalar1=-2147483648, scalar2=None,
                            op0=ALU.bitwise_and)
    nc.vector.tensor_tensor(out=h.bitcast(I32), in0=h.bitcast(I32),
                            in1=sbit, op=ALU.bitwise_or)
    g = sb_tmp.tile([128, QW], F32, tag="qt_g", name="qt_g")[:P, :W]
    nc.vector._custom_dve(FP4_LO_MERGE, out=g, in0=y, in1=h,
                          s0=MAGIC, s1=6.25, imm2=-2.0)
    nc.vector.tensor_tensor(
        out=out_ap.rearrange("p (b s) -> p b s", s=16),
        in0=g.rearrange("p (b s) -> p b s", s=16),
        in1=scale_ap.unsqueeze(2).broadcast_to([P, nb, 16]),
        op=ALU.mult)


def _amax_scales(nc, sb_tmp, in_ap, W, P=128):
    """Returns (scale, rs6) [P, W//16] tiles for fp4 quant of in_ap."""
    nb = W // 16
    amax = sb_tmp.tile([128, 64], F32, tag="am", name="am")[:P, :nb]
    nc.vector.tensor_reduce(amax, in_ap.rearrange("p (b s) -> p b s", s=16),
                            axis=mybir.AxisListType.X, op=ALU.max,
                            apply_absolute_value=True)
    amc = sb_tmp.tile([128, 64], F32, tag="ac", name="ac")[:P, :nb]
    nc.vector.tensor_scalar_max(amc, amax, 1e-30)
    rcp = sb_tmp.tile([128, 64], F32, tag="rc", name="rc")[:P, :nb]
    nc.vector.reciprocal(rcp, amc)
    rs6 = sb_tmp.tile([128, 64], F32, tag="r6", name="r6")[:P, :nb]
    nc.vector.tensor_scalar_mul(rs6, rcp, 6.0)
    scale = sb_tmp.tile([128, 64], F32, tag="sc", name="sc")[:P, :nb]
    nc.vector.tensor_scalar_mul(scale, amax, 1.0 / 6.0)
    return scale, rs6, amax


def build():
    nc = bacc.Bacc("TRN2", target_bir_lowering=False, debug=False,
                   num_devices=NCORES)
    xq_d = nc.dram_tensor("xqT", [16, 128, T], F16,
                          kind="ExternalInput").ap()  # host-prequantized x^T
    wq_d = nc.dram_tensor("wqT", [16, 128, OD], F16,
                          kind="ExternalInput").ap()
    wk_d = nc.dram_tensor("wkT", [16, 128, OD], F16,
                          kind="ExternalInput").ap()
    wv_d = nc.dram_tensor("wvT", [16, 128, OD], F16,
                          kind="ExternalInput").ap()
    wo_d = nc.dram_tensor("woT", [2, 128, HID], BF16,
                          kind="ExternalInput").ap()
    cos_d = nc.dram_tensor("cosT", [128, T], F32, kind="ExternalInput").ap()
    sin_d = nc.dram_tensor("sinTs", [128, T], F32, kind="ExternalInput").ap()
    mask_d = nc.dram_tensor("masks", [128, 256], F32,
                            kind="ExternalInput").ap()
    out_d = nc.dram_tensor("partialT", [HID, T], BF16,
                           kind="ExternalOutput").ap()

    with tile.TileContext(nc) as tc, ExitStack() as ctx:
        sb_w = ctx.enter_context(tc.tile_pool(name="sb_w", bufs=1))
        sb_tmp = ctx.enter_context(tc.tile_pool(name="sb_tmp", bufs=1))
        sb_io = ctx.enter_context(tc.tile_pool(name="sb_io", bufs=2))
        sb_att = ctx.enter_context(tc.tile_pool(name="sb_att", bufs=1))
        sb_pt = ctx.enter_context(tc.tile_pool(name="sb_pt", bufs=2))
        # PSUM: 8 banks total.  Separate tags per stream so batch-b1
        # projections can overlap batch-b0 attention without pool-slot
        # serialization: pj(2, shared with o_proj) + sc(3) + oT(2) + tr(1) = 8.
        ps_pj = ctx.enter_context(
            tc.tile_pool(name="ps_pj", bufs=2, space="PSUM"))
        ps_sc = ctx.enter_context(
            tc.tile_pool(name="ps_sc", bufs=2, space="PSUM"))
        ps_po = ctx.enter_context(
            tc.tile_pool(name="ps_po", bufs=1, space="PSUM"))
        ps_ot = ctx.enter_context(
            tc.tile_pool(name="ps_ot", bufs=2, space="PSUM"))
        ps_tr = ctx.enter_context(
            tc.tile_pool(name="ps_tr", bufs=1, space="PSUM"))

        ident = sb_w.tile([128, 128], F32)
        make_identity(nc, ident[:])
        masksF = sb_w.tile([128, 256], F32)
        nc.sync.dma_start(masksF[:], mask_d)

        def quant_rows(dst_ap, src_ap, W):
            """quantize src [128, W] into dst, splitting into QW pieces."""
            for off in range(0, W, QW):
                w = min(QW, W - off)
                scale, rs6, _ = _amax_scales(nc, sb_tmp,
                                             src_ap[:, off:off + w], w)
                _quant(nc, sb_tmp, dst_ap[:, off:off + w],
                       src_ap[:, off:off + w], scale, rs6, w)


        # --------- weights: pre-quantized + transposed on host ---------
        wT = {}
        for nm, wd in (("q", wq_d), ("k", wk_d), ("v", wv_d)):
            wt = sb_w.tile([128, 16 * OD], F16, name=f"w{nm}T")
            wT[nm] = wt
            nc.sync.dma_start(wt[:].rearrange("p (a t) -> p a t", a=16),
                              wd.rearrange("a p t -> p a t"))
        woT = sb_w.tile([128, 2 * HID], BF16, name="woT")

        def wo_prep():
            for a in range(2):
                nc.sync.dma_start(woT[:, a * HID:(a + 1) * HID], wo_d[a])

        # persistent per-batch attention buffers (double-buffered across
        # batches so b1 projections overlap b0 attention)
        qT = {b: [sb_att.tile([128, S], F16, name=f"qT{b}{m}")
                  for m in range(2)] for b in range(B)}
        kT = {b: [sb_att.tile([128, S], F16, name=f"kT{b}{m}")
                  for m in range(2)] for b in range(B)}
        vE = {b: [sb_att.tile([128, 16 * 65], F16, name=f"vE{b}{h}")
                  for h in range(HPC)] for b in range(B)}
        oqT = {b: sb_att.tile([128, 2 * S], BF16, name=f"oqT{b}")
               for b in range(B)}

        NCH = S // TC  # chunks per batch (4)

        def rope_piece(b, pc):
            t0 = b * S
            for dst in (qT[b], kT[b]):
                for m in range(2):
                    c0 = pc * 512
                    cosT = sb_io.tile([128, 512], F32, tag="rope_c", bufs=1)
                    sinT = sb_io.tile([128, 512], F32, tag="rope_s", bufs=1)
                    nc.sync.dma_start(cosT[:],
                                      cos_d[:, t0 + c0:t0 + c0 + 512])
                    nc.sync.dma_start(sinT[:],
                                      sin_d[:, t0 + c0:t0 + c0 + 512])
                    sh = sb_io.tile([128, 512], F16, tag="rope_sh", bufs=1)
                    for hh in range(2):
                        p0 = hh * 64
                        nc.sync.dma_start(
                            sh[p0:p0 + 32, :],
                            dst[m][p0 + 32:p0 + 64, c0:c0 + 512])
                        nc.sync.dma_start(
                            sh[p0 + 32:p0 + 64, :],
                            dst[m][p0:p0 + 32, c0:c0 + 512])
                    tcos = sb_io.tile([128, 512], F32, tag="rope_tc", bufs=1)
                    shs = sb_io.tile([128, 512], F32, tag="rope_ss", bufs=1)
                    nc.vector.tensor_tensor(
                        out=tcos[:], in0=dst[m][:, c0:c0 + 512],
                        in1=cosT[:], op=ALU.mult)
                    nc.vector.tensor_tensor(out=shs[:], in0=sh[:],
                                            in1=sinT[:], op=ALU.mult)
                    nc.vector.tensor_tensor(
                        out=dst[m][:, c0:c0 + 512], in0=tcos[:],
                        in1=shs[:], op=ALU.add)

        def proj_chunk(b, cchunk):
            cc0 = cchunk * TC
            xqT = sb_pt.tile([128, 16 * TC], F16, tag="xqT", name="xqT",
                             bufs=3)
            nc.sync.dma_start(
                xqT[:].rearrange("p (a t) -> p a t", a=16),
                xq_d[:, :, b * S + cc0: b * S + cc0 + TC]
                .rearrange("a p t -> p a t"))
            for nm in ("q", "k", "v"):
                for m in range(2):
                    pj = ps_pj.tile([128, TC], F32, tag="pj")
                    for i in range(16):
                        nc.tensor.matmul(
                            pj[:],
                            wT[nm][:, i * OD + m * 128:
                                   i * OD + (m + 1) * 128],
                            xqT[:, i * TC:(i + 1) * TC],
                            start=(i == 0), stop=(i == 15))
                    if nm == "v":
                        # to v-natural tiles with a ones column
                        vsb = sb_io.tile([128, TC], F32, tag="vsb")
                        nc.vector.tensor_copy(vsb[:], pj[:])
                        for hh in range(2):
                            h_ = m * 2 + hh
                            ptv4 = ps_tr.tile([128, 256], F32, tag="ps_tr")
                            for kt in range(TC // 128):
                                nc.tensor.transpose(
                                    ptv4[:, kt * 64:(kt + 1) * 64],
                                    vsb[hh * 64:(hh + 1) * 64,
                                        kt * 128:(kt + 1) * 128],
                                    ident[hh * 64:(hh + 1) * 64,
                                          hh * 64:(hh + 1) * 64])
                            k0 = (cc0 // 128)
                            dstv = vE[b][h_][:, k0 * 65:(k0 + 4) * 65] \
                                .rearrange("p (a t) -> p a t", t=65)
                            nc.vector.tensor_copy(
                                dstv[:, :, 0:64],
                                ptv4[:].rearrange("p (a t) -> p a t", a=4))
                            nc.vector.memset(
                                dstv[:, :, 64:65], 1.0)
                    else:
                        dst = qT[b][m] if nm == "q" else kT[b][m]
                        nc.scalar.copy(dst[:, cc0:cc0 + TC], pj[:])
            rope_piece(b, cchunk)

        def attention_qc(b, qc):
            # scores transposed: sT[k, q]; qc outer so o-quant batches
            # all 4 heads into [128, 256] pieces
            onat = sb_io.tile([128, 4 * 256], F32, tag="onat", bufs=2,
                              name="onat")
            rsum = sb_io.tile([128, 16], F32, tag="rsum", name="rsum")
            last = 4 * qc + 3
            for m in range(2):
                oTq2 = [ps_ot.tile([65, 512], F32, tag="ps_oT",
                                   name="ps_oT") for _ in range(2)]
                for kblk in range(4 * qc + 4):
                    qs0 = max(qc * 512, kblk * 128)
                    # pad diagonal blocks to >=256 wide (f32r/fp16
                    # matmuls need N>=256 for 1 cyc/row; padded cols
                    # are fully masked -> exp=0 -> PV adds zeros)
                    qs = min(qs0, (qc + 1) * 512 - 256)
                    w = (qc + 1) * 512 - qs
                    off = qs - qc * 512
                    pad = qs < qs0
                    # both heads of this m-tile back-to-back: their
                    # K=64 row groups (base 0 / 64) run concurrently
                    is_diag = kblk >= 4 * qc
                    scs = []
                    for hh in range(2):
                        p0 = hh * 64
                        sc = ps_sc.tile([128, 512], F32, tag="sc",
                                        name="sc")
                        scs.append(sc)
                        nc.tensor.matmul(
                            sc[:, 0:w],
                            kT[b][m][p0:p0 + 64,
                                     kblk * 128:(kblk + 1) * 128],
                            qT[b][m][p0:p0 + 64, qs:(qc + 1) * 512],
                            start=True, stop=True)
                    for hh in range(2):
                        h_ = m * 2 + hh
                        sc = scs[hh]
                        if is_diag:
                            if pad:
                                nc.vector.tensor_tensor(
                                    out=sc[:, 0:256], in0=sc[:, 0:256],
                                    in1=masksF[:, 0:256], op=ALU.add)
                            else:
                                nc.vector.tensor_tensor(
                                    out=sc[:, 0:128], in0=sc[:, 0:128],
                                    in1=masksF[:, 128:256], op=ALU.add)
                        pT = sb_pt.tile([128, 512], F16, tag="pT",
                                        name="pT", bufs=4)
                        nc.scalar.activation(pT[:, 0:w], sc[:, 0:w],
                                             ACTF.Exp, scale=0.125)
                        nc.tensor.matmul(
                            oTq2[hh][:, off:off + w],
                            vE[b][h_][:, kblk * 65:(kblk + 1) * 65],
                            pT[:, 0:w],
                            start=(kblk == 0),
                            stop=(kblk == last),
                            skip_group_check=(kblk == last and off != 0))
                # evacuate both heads: batch the 4 tt transposes
                for hh in range(2):
                    h_ = m * 2 + hh
                    osb = sb_io.tile([128, 512], F32, tag="osb",
                                     name="osb", bufs=2)
                    nc.vector.tensor_copy(osb[0:65, :], oTq2[hh][:])
                    ptn4 = ps_tr.tile([128, 260], F32, tag="ps_tr",
                                      name="ptn4")
                    for tt in range(4):
                        nc.tensor.transpose(
                            ptn4[:, tt * 65: tt * 65 + 65],
                            osb[0:65, tt * 128:(tt + 1) * 128],
                            ident[0:65, 0:65])
                    src4 = ptn4[:].rearrange("p (a t) -> p a t", t=65)
                    nc.vector.tensor_copy(
                        onat[:].rearrange("p (a t) -> p a t", a=4)
                        [:, :, h_ * 64:(h_ + 1) * 64],
                        src4[:, :, 0:64])
                    nc.vector.reciprocal(
                        rsum[:].rearrange("p (a t) -> p a t", t=4)
                        [:, :, h_:h_ + 1],
                        src4[:, :, 64:65])
            # quantize [128, 256] pieces (4 heads wide), fold 1/sum
            for tt in range(4):
                seg = onat[:, tt * 256:(tt + 1) * 256]
                amax = sb_tmp.tile([128, 64], F32, tag="am",
                                   name="am")[:, 0:16]
                nc.vector.tensor_reduce(
                    amax, seg.rearrange("p (b s) -> p b s", s=16),
                    axis=mybir.AxisListType.X, op=ALU.max,
                    apply_absolute_value=True)
                amc = sb_tmp.tile([128, 64], F32, tag="ac",
                                  name="ac")[:, 0:16]
                nc.vector.tensor_scalar_max(amc, amax, 1e-30)
                rcp = sb_tmp.tile([128, 64], F32, tag="rc",
                                  name="rc")[:, 0:16]
                nc.vector.reciprocal(rcp, amc)
                rs6 = sb_tmp.tile([128, 64], F32, tag="r6",
                                  name="r6")[:, 0:16]
                nc.vector.tensor_scalar_mul(rs6, rcp, 6.0)
                sct = sb_tmp.tile([128, 64], F32, tag="sc",
                                  name="sct")[:, 0:16]
                nc.vector.tensor_tensor(
                    out=sct.rearrange("p (h s) -> p h s", s=4),
                    in0=amax.rearrange("p (h s) -> p h s", s=4),
                    in1=rsum[:, tt * 4:(tt + 1) * 4].unsqueeze(2)
                    .broadcast_to([128, 4, 4]),
                    op=ALU.mult)
                nc.vector.tensor_scalar_mul(sct, sct, 1.0 / 6.0)
                oq = sb_io.tile([128, 256], F32, tag="oq", name="oq",
                                bufs=2)
                _quant(nc, sb_tmp, oq[:], seg, sct, rs6, 256)
                # transpose into oqT: cols h*64.. go to rows mm*128..
                tglob = qc * 4 + tt
                ptq2 = ps_tr.tile([128, 256], F32, tag="ps_tr",
                                  name="ptq2")
                for mm in range(2):
                    nc.tensor.transpose(
                        ptq2[:, mm * 128:(mm + 1) * 128],
                        oq[:, mm * 128:(mm + 1) * 128], ident[:])
                nc.vector.tensor_copy(
                    oqT[b][:].rearrange("p (a t) -> p a t", a=2)
                    [:, :, tglob * 128:(tglob + 1) * 128],
                    ptq2[:].rearrange("p (a t) -> p a t", a=2))

        def oproj_chunk(b, tch):
            t0 = b * S
            tc0 = tch * 512
            for mo in range(16):
                po = ps_po.tile([128, 512], F32, tag="po")
                for i in range(2):
                    nc.tensor.matmul(
                        po[:],
                        woT[:, i * HID + mo * 128:
                            i * HID + (mo + 1) * 128],
                        oqT[b][:, i * S + tc0: i * S + tc0 + 512],
                        start=(i == 0), stop=(i == 1))
                posb = sb_io.tile([128, 512], BF16, tag="posb",
                                  name="posb")
                if b == 0 and mo % 2 == 0:
                    nc.scalar.copy(posb[:], po[:])
                else:
                    nc.vector.tensor_copy(posb[:], po[:])
                nc.sync.dma_start(
                    out_d[mo * 128:(mo + 1) * 128,
                          t0 + tc0:t0 + tc0 + 512],
                    posb[:])

        # ---- batch 0 projections; x_prep(1)/wo interleaved ----
        wo_prep()
        for cc in range(NCH):
            proj_chunk(0, cc)
        # ---- b0 attention interleaved with b1 projections; b0's o_proj
        # is deferred into the tail (PE filler while b1 attention keeps
        # the ACT engine busy); b1 attention runs longest-qc first so the
        # final serial drain is the shortest qc ----
        for qc in range(4):
            attention_qc(0, qc)
            oproj_chunk(0, qc)
            proj_chunk(1, qc)
        for qc in range(4):
            attention_qc(1, qc)
            oproj_chunk(1, qc)

    nc.compile()
    return nc


def _np_quant(x):
    """Host fp4 fake-quant, bit-exact to the device implementation."""
    sh = x.shape
    xb = x.reshape(sh[:-1] + (sh[-1] // 16, 16)).astype(np.float32)
    amax = np.max(np.abs(xb), axis=-1, keepdims=True).astype(np.float32)
    amax_c = np.maximum(amax, np.float32(1e-30))
    rcp = (np.float32(1.0) / amax_c).astype(np.float32)
    rs6 = (rcp * np.float32(6.0)).astype(np.float32)
    scale = (amax * np.float32(1.0 / 6.0)).astype(np.float32)
    y = (xb * rs6).astype(np.float32)
    yi = y.view(np.int32)
    rem = yi & 0x3FFFFF
    inc = (rem > 0x200000).astype(np.int32) << 22
    h = ((yi & np.int32(-4194304)) + inc).view(np.float32)
    M32 = np.float32(MAGIC)
    low = ((y + M32).astype(np.float32) - M32).astype(np.float32)
    q = np.where(np.abs(y) > np.float32(2.0), h, low)
    return (q * scale).astype(np.float32).reshape(sh)


_HOST_CACHE = {}


def _host_tables():
    if _HOST_CACHE:
        return _HOST_CACHE
    D = HD
    inv = (1.0 / (10000.0 ** (np.arange(0, D, 2, dtype=np.float32)
                              / np.float32(D)))).astype(np.float32)
    fr = (np.arange(S, dtype=np.float32)[:, None] * inv[None, :]).astype(
        np.float32)
    cos = np.concatenate([np.cos(fr), np.cos(fr)], -1).astype(np.float32)
    sin = np.concatenate([np.sin(fr), np.sin(fr)], -1).astype(np.float32)
    cosT = np.zeros((128, T), np.float32)
    sinTs = np.zeros((128, T), np.float32)
    sgn = np.where(np.arange(D) < D // 2, np.float32(-1.0), np.float32(1.0))
    for bb in range(B):
        cosT[:, bb * S:(bb + 1) * S] = np.tile(cos.T, (2, 1))
        sinTs[:, bb * S:(bb + 1) * S] = np.tile((sin * sgn[None, :]).T,
                                                (2, 1))
    # mask table [128, 256], sT layout: col j (global q = qs + j), row k:
    # masked (NEG) iff (j - 128) < k.  cols 0-127: fully masked (used for
    # padded diagonal blocks); cols 128-255: the standard triangle.
    masks = np.zeros((128, 256), np.float32)
    for kk in range(128):
        masks[kk, :128 + kk] = NEG
    _HOST_CACHE.update(cosT=cosT, sinTs=sinTs, masks=masks)
    return _HOST_CACHE


_NC_CACHE = []


def make_in_maps(hidden_states, Wq, Wk, Wv, Wo):
    tabs = _host_tables()
    xf = hidden_states.reshape(T, HID)
    xq16 = np.ascontiguousarray(
        _np_quant(np.asarray(xf, np.float32)).T.reshape(16, 128, T)
        .astype(np.float16))
    wq_q = _np_quant(np.asarray(Wq, np.float32))
    wk_q = _np_quant(np.asarray(Wk, np.float32))
    wv_q = _np_quant(np.asarray(Wv, np.float32))
    wo_q = _np_quant(np.asarray(Wo, np.float32))
    import ml_dtypes
    bf16 = ml_dtypes.bfloat16
    in_maps = []
    for c in range(NCORES):
        sl = slice(c * OD, (c + 1) * OD)
        wqT = np.ascontiguousarray(
            wq_q[sl, :].T.reshape(16, 128, OD).astype(np.float16))
        wkT = np.ascontiguousarray(
            wk_q[sl, :].T.reshape(16, 128, OD).astype(np.float16))
        wvT = np.ascontiguousarray(
            wv_q[sl, :].T.reshape(16, 128, OD).astype(np.float16))
        woTc = np.ascontiguousarray(
            wo_q[:, sl].T.reshape(2, 128, HID).astype(bf16))
        in_maps.append(dict(
            xqT=xq16,
            wqT=wqT, wkT=wkT, wvT=wvT, woT=woTc,
            cosT=tabs['cosT'], sinTs=tabs['sinTs'], masks=tabs['masks'],
        ))
    return in_maps


def kernel(hidden_states, Wq, Wk, Wv, Wo):
    in_maps = make_in_maps(hidden_states, Wq, Wk, Wv, Wo)
    if not _NC_CACHE:
        _NC_CACHE.append(build())
    nc = _NC_CACHE[0]
    res = bass_utils.run_bass_kernel_spmd(nc, in_maps,
                                          core_ids=list(range(NCORES)))
    total = np.zeros((HID, T), np.float32)
    for r in res.results:
        total += np.asarray(r["partialT"], dtype=np.float32)
    return np.ascontiguousarray(total.T.reshape(B, S, HID))


if __name__ == "__main__":
    d = np.load('/root/problem/inputs.npz')
    out = kernel(d['hidden_states'], d['Wq'], d['Wk'], d['Wv'], d['Wo'])
    ref = np.load('/root/problem/ref_out.npy')
    rel2 = np.linalg.norm(out - ref) / np.linalg.norm(ref)
    print(f"relL2={rel2:.3e} absmax={np.abs(out - ref).max():.3e}")


# revision 27
# speedup vs baseline: 1.1039x; 1.1039x over previous
"""Trainium2 Bass kernel for AttentionWithFP4Projections.

Sharding: tensor-parallel over heads across 8 cores (4 heads each, both
batches). Each core computes q/k/v for its 256 output dims, full causal
attention for its heads, and a partial o_proj (its 256-dim slice of the
contraction); partials are summed on the host (no device collectives).

Numerics: FP4 fake-quant reproduced bit-exactly (magic-constant rounding
within fp32-ALU-exact ranges), split DVE (mults/select) + GpSimd (bit ops).
Matmul dtypes chosen for PE speed (1 cyc/row instead of fp32's 4):
  - q/k/v projections + scores: fp16 operands (quantized values rounded
    to fp16; ~5e-4 rel, fine through softmax),
  - PV (o accumulation): float32r (fp22) - o feeds the o-quant whose
    bucket decisions amplify pre-quant error, so keep it >= fp22,
  - o_proj: bf16 (post-quant, no amplification).
Softmax without max-subtraction (max scaled score ~5, no overflow);
normalization folded into the o-quant scale via a ones-column in V.
"""
import sys
import types
from contextlib import ExitStack

import numpy as np

# The NTFF profiling hook module is missing in this image; shim it so
# run_bass_kernel_spmd(trace=True) works (used by test.py, harmless here).
if 'antenv.axon_hooks' not in sys.modules:
    _m = types.ModuleType('antenv.axon_hooks')
    _m._hook = None
    _m.set_axon_ntff_profile_hook = lambda h: setattr(_m, '_hook', h)
    _m.get_axon_ntff_profile_hook = lambda: _m._hook
    sys.modules['antenv.axon_hooks'] = _m
    try:
        from trn_agent_boot.trn_boot import _ntff_profile_via_ctypes
        _m._hook = _ntff_profile_via_ctypes('/opt/axon/libaxon_pjrt.so')
    except Exception:
        pass

import concourse.mybir as mybir
import concourse.tile as tile
from concourse import bacc
from concourse import bass_utils
from concourse.masks import make_identity

# ---- custom fused DVE ops for the FP4 grid rounding ----------------------
# Extends the module-level registry (the documented add-an-op flow; the
# dve_ops.py file itself is read-only in this image).  Rows 17/18 are free
# (16 stock ops occupy 1..16; the byte-36 row field allows [1, 0x20)).
import concourse.dve_ops as _dvo
from concourse.dve_spec import (Spec as _Spec, Src0 as _Src0, Src1 as _Src1,
                                C0 as _C0, C1 as _C1, C2 as _C2,
                                Zero as _Zero, One as _One,
                                maxx as _maxx, minn as _minn, sq as _sq)


def _register_fp4_ops():
    import numpy as _np
    by_name = {o.name: o for o in _dvo.OPS}
    if "FP4_HI_ANT" in by_name:
        return by_name["FP4_HI_ANT"], by_name["FP4_LO_MERGE_ANT"]
    _ysq = _sq(_Src0)
    _i1 = _C0 < _ysq   # s0 = 2.5**2
    _i2 = _C1 < _ysq   # s1 = 3.5**2
    _i3 = _C2 < _ysq   # imm2 = 5**2
    # habs = 3*(|y|>2.5) + (|y|>3.5) + 2*(|y|>5), via (i1+i3)*2 + i2 + i1
    specB = _Spec(
        body=(_i1 + _i3) * (_One + _One) + _i2 + _i1,
        reference=lambda in0, s0, s1, imm2: (
            ((in0 * in0 > s0).astype(_np.float32) + (in0 * in0 > imm2)) * 2.0
            + (in0 * in0 > s1) + (in0 * in0 > s0)),
    )
    # g = clamp(magic_round(y), -2, 2) * (y*y <= 6.25) + in1(signed high)
    specA = _Spec(
        body=_minn(_maxx((_Src0 + _C0) - _C0, _C2), _Zero - _C2)
        * (_sq(_Src0) <= _C1) + _Src1,
        reference=lambda in0, in1, s0, s1, imm2: _np.clip(
            (in0 + _np.float32(s0)).astype(_np.float32) - _np.float32(s0),
            imm2, -imm2) * (in0 * in0 <= s1) + in1,
    )
    opB = _dvo.DveOp("FP4_HI_ANT", specB, subdim=False,
                     uops_sha={"v3": "176720cc7ee0a7f8",
                               "v4": "014accfcba4ba70e"})
    opA = _dvo.DveOp("FP4_LO_MERGE_ANT", specA, subdim=False,
                     uops_sha={"v3": "b57d557c01bd412c",
                               "v4": "780a6585d0fe9dbb"})
    _dvo.OPS.extend([opB, opA])
    _dvo._SUB_OPCODE_FOR_NAME[opB.name] = 17
    _dvo._SUB_OPCODE_FOR_NAME[opA.name] = 18
    _dvo.CUSTOM_DVE_SPECS[opB.name] = specB
    _dvo.CUSTOM_DVE_SPECS[opA.name] = specA
    return opB, opA


FP4_HI, FP4_LO_MERGE = _register_fp4_ops()

F32 = mybir.dt.float32
F32R = mybir.dt.float32r
F16 = mybir.dt.float16
BF16 = mybir.dt.bfloat16
I32 = mybir.dt.int32
ALU = mybir.AluOpType
ACTF = mybir.ActivationFunctionType

NCORES = 8
B, S, HID = 2, 2048, 2048
T = B * S                     # 4096 tokens
NH, HD = 32, 64               # heads, head dim
HPC = NH // NCORES            # 4 heads per core
OD = HPC * HD                 # 256 output dims per core
SPC = S // NCORES             # 256 tokens per batch per core
TC = 512                      # token-chunk width for projections
QW = 512                      # quantization sub-width (temp buffer size)
MAGIC = 6291456.0             # 1.5*2^22: +/- rounds fp32 to multiples of 0.5
NEG = -1.0e30


def _quant(nc, sb_tmp, out_ap, in_ap, scale_ap, rs6_ap, W, P=128):
    """FP4 fake-quant of in_ap [P, W] -> out_ap, given per-block scale and
    rs6 (=6/amax) [P, W//16].  Matches the jnp reference up to 1-ulp
    boundary/tie cases (reciprocal-based scale path, squared-bound
    compares).  Uses two fused custom DVE ops: 6 big passes total."""
    nb = W // 16
    y = sb_tmp.tile([128, QW], F32, tag="qt_y", name="qt_y")[:P, :W]
    nc.vector.tensor_tensor(
        out=y.rearrange("p (b s) -> p b s", s=16),
        in0=in_ap.rearrange("p (b s) -> p b s", s=16),
        in1=rs6_ap.unsqueeze(2).broadcast_to([P, nb, 16]),
        op=ALU.mult)
    h = sb_tmp.tile([128, QW], F32, tag="qt_h", name="qt_h")[:P, :W]
    nc.vector._custom_dve(FP4_HI, out=h, in0=y,
                          s0=6.25, s1=12.25, imm2=25.0)
    sbit = sb_tmp.tile([128, QW], I32, tag="qt_s", name="qt_s")[:P, :W]
    nc.vector.tensor_scalar(out=sbit, in0=in_ap.bitcast(I32),
                            scalar1=-2147483648, scalar2=None,
                            op0=ALU.bitwise_and)
    nc.vector.tensor_tensor(out=h.bitcast(I32), in0=h.bitcast(I32),
                            in1=sbit, op=ALU.bitwise_or)
    g = sb_tmp.tile([128, QW], F32, tag="qt_g", name="qt_g")[:P, :W]
    nc.vector._custom_dve(FP4_LO_MERGE, out=g, in0=y, in1=h,
                          s0=MAGIC, s1=6.25, imm2=-2.0)
    nc.vector.tensor_tensor(
        out=out_ap.rearrange("p (b s) -> p b s", s=16),
        in0=g.rearrange("p (b s) -> p b s", s=16),
        in1=scale_ap.unsqueeze(2).broadcast_to([P, nb, 16]),
        op=ALU.mult)


def _amax_scales(nc, sb_tmp, in_ap, W, P=128):
    """Returns (scale, rs6) [P, W//16] tiles for fp4 quant of in_ap."""
    nb = W // 16
    amax = sb_tmp.tile([128, 64], F32, tag="am", name="am")[:P, :nb]
    nc.vector.tensor_reduce(amax, in_ap.rearrange("p (b s) -> p b s", s=16),
                            axis=mybir.AxisListType.X, op=ALU.max,
                            apply_absolute_value=True)
    amc = sb_tmp.tile([128, 64], F32, tag="ac", name="ac")[:P, :nb]
    nc.vector.tensor_scalar_max(amc, amax, 1e-30)
    rcp = sb_tmp.tile([128, 64], F32, tag="rc", name="rc")[:P, :nb]
    nc.vector.reciprocal(rcp, amc)
    rs6 = sb_tmp.tile([128, 64], F32, tag="r6", name="r6")[:P, :nb]
    nc.vector.tensor_scalar_mul(rs6, rcp, 6.0)
    scale = sb_tmp.tile([128, 64], F32, tag="sc", name="sc")[:P, :nb]
    nc.vector.tensor_scalar_mul(scale, amax, 1.0 / 6.0)
    return scale, rs6, amax


def build():
    nc = bacc.Bacc("TRN2", target_bir_lowering=False, debug=False,
                   num_devices=NCORES)
    xq_d = nc.dram_tensor("xqT", [16, 128, T], F16,
                          kind="ExternalInput").ap()  # host-prequantized x^T
    wq_d = nc.dram_tensor("wqT", [16, 128, OD], F16,
                          kind="ExternalInput").ap()
    wk_d = nc.dram_tensor("wkT", [16, 128, OD], F16,
                          kind="ExternalInput").ap()
    wv_d = nc.dram_tensor("wvT", [16, 128, OD], F16,
                          kind="ExternalInput").ap()
    wo_d = nc.dram_tensor("woT", [2, 128, HID], BF16,
                          kind="ExternalInput").ap()
    cos_d = nc.dram_tensor("cosT", [128, T], F32, kind="ExternalInput").ap()
    sin_d = nc.dram_tensor("sinTs", [128, T], F32, kind="ExternalInput").ap()
    mask_d = nc.dram_tensor("masks", [128, 256], F32,
                            kind="ExternalInput").ap()
    out_d = nc.dram_tensor("partialT", [HID, T], BF16,
                           kind="ExternalOutput").ap()

    with tile.TileContext(nc) as tc, ExitStack() as ctx:
        sb_w = ctx.enter_context(tc.tile_pool(name="sb_w", bufs=1))
        sb_tmp = ctx.enter_context(tc.tile_pool(name="sb_tmp", bufs=1))
        sb_io = ctx.enter_context(tc.tile_pool(name="sb_io", bufs=2))
        sb_att = ctx.enter_context(tc.tile_pool(name="sb_att", bufs=1))
        sb_pt = ctx.enter_context(tc.tile_pool(name="sb_pt", bufs=2))
        # PSUM: 8 banks total.  Separate tags per stream so batch-b1
        # projections can overlap batch-b0 attention without pool-slot
        # serialization: pj(2, shared with o_proj) + sc(3) + oT(2) + tr(1) = 8.
        ps_pj = ctx.enter_context(
            tc.tile_pool(name="ps_pj", bufs=2, space="PSUM"))
        ps_sc = ctx.enter_context(
            tc.tile_pool(name="ps_sc", bufs=2, space="PSUM"))
        ps_po = ctx.enter_context(
            tc.tile_pool(name="ps_po", bufs=1, space="PSUM"))
        ps_ot = ctx.enter_context(
            tc.tile_pool(name="ps_ot", bufs=2, space="PSUM"))
        ps_tr = ctx.enter_context(
            tc.tile_pool(name="ps_tr", bufs=1, space="PSUM"))

        ident = sb_w.tile([128, 128], F32)
        make_identity(nc, ident[:])
        masksF = sb_w.tile([128, 256], F32)
        nc.sync.dma_start(masksF[:], mask_d)

        def quant_rows(dst_ap, src_ap, W):
            """quantize src [128, W] into dst, splitting into QW pieces."""
            for off in range(0, W, QW):
                w = min(QW, W - off)
                scale, rs6, _ = _amax_scales(nc, sb_tmp,
                                             src_ap[:, off:off + w], w)
                _quant(nc, sb_tmp, dst_ap[:, off:off + w],
                       src_ap[:, off:off + w], scale, rs6, w)


        # --------- weights: pre-quantized + transposed on host ---------
        wT = {}
        for nm, wd in (("q", wq_d), ("k", wk_d), ("v", wv_d)):
            wt = sb_w.tile([128, 16 * OD], F16, name=f"w{nm}T")
            wT[nm] = wt
            nc.sync.dma_start(wt[:].rearrange("p (a t) -> p a t", a=16),
                              wd.rearrange("a p t -> p a t"))
        woT = sb_w.tile([128, 2 * HID], BF16, name="woT")

        def wo_prep():
            for a in range(2):
                nc.sync.dma_start(woT[:, a * HID:(a + 1) * HID], wo_d[a])

        # persistent per-batch attention buffers (double-buffered across
        # batches so b1 projections overlap b0 attention)
        qT = {b: [sb_att.tile([128, S], F16, name=f"qT{b}{m}")
                  for m in range(2)] for b in range(B)}
        kT = {b: [sb_att.tile([128, S], F16, name=f"kT{b}{m}")
                  for m in range(2)] for b in range(B)}
        vE = {b: [sb_att.tile([128, 16 * 65], F16, name=f"vE{b}{h}")
                  for h in range(HPC)] for b in range(B)}
        oqT = {b: sb_att.tile([128, 2 * S], BF16, name=f"oqT{b}")
               for b in range(B)}

        NCH = S // TC  # chunks per batch (4)

        def rope_piece(b, pc):
            t0 = b * S
            for dst in (qT[b], kT[b]):
                for m in range(2):
                    c0 = pc * 512
                    cosT = sb_io.tile([128, 512], F32, tag="rope_c", bufs=1)
                    sinT = sb_io.tile([128, 512], F32, tag="rope_s", bufs=1)
                    nc.sync.dma_start(cosT[:],
                                      cos_d[:, t0 + c0:t0 + c0 + 512])
                    nc.sync.dma_start(sinT[:],
                                      sin_d[:, t0 + c0:t0 + c0 + 512])
                    sh = sb_io.tile([128, 512], F16, tag="rope_sh", bufs=1)
                    for hh in range(2):
                        p0 = hh * 64
                        nc.sync.dma_start(
                            sh[p0:p0 + 32, :],
                            dst[m][p0 + 32:p0 + 64, c0:c0 + 512])
                        nc.sync.dma_start(
                            sh[p0 + 32:p0 + 64, :],
                            dst[m][p0:p0 + 32, c0:c0 + 512])
                    tcos = sb_io.tile([128, 512], F32, tag="rope_tc", bufs=1)
                    shs = sb_io.tile([128, 512], F32, tag="rope_ss", bufs=1)
                    nc.vector.tensor_tensor(
                        out=tcos[:], in0=dst[m][:, c0:c0 + 512],
                        in1=cosT[:], op=ALU.mult)
                    nc.vector.tensor_tensor(out=shs[:], in0=sh[:],
                                            in1=sinT[:], op=ALU.mult)
                    nc.vector.tensor_tensor(
                        out=dst[m][:, c0:c0 + 512], in0=tcos[:],
                        in1=shs[:], op=ALU.add)

        def proj_chunk(b, cchunk):
            cc0 = cchunk * TC
            xqT = sb_pt.tile([128, 16 * TC], F16, tag="xqT", name="xqT")
            nc.sync.dma_start(
                xqT[:].rearrange("p (a t) -> p a t", a=16),
                xq_d[:, :, b * S + cc0: b * S + cc0 + TC]
                .rearrange("a p t -> p a t"))
            for nm in ("q", "k", "v"):
                for m in range(2):
                    pj = ps_pj.tile([128, TC], F32, tag="pj")
                    for i in range(16):
                        nc.tensor.matmul(
                            pj[:],
                            wT[nm][:, i * OD + m * 128:
                                   i * OD + (m + 1) * 128],
                            xqT[:, i * TC:(i + 1) * TC],
                            start=(i == 0), stop=(i == 15))
                    if nm == "v":
                        # to v-natural tiles with a ones column
                        vsb = sb_io.tile([128, TC], F32, tag="vsb")
                        nc.vector.tensor_copy(vsb[:], pj[:])
                        for hh in range(2):
                            h_ = m * 2 + hh
                            ptv4 = ps_tr.tile([128, 256], F32, tag="ps_tr")
                            for kt in range(TC // 128):
                                nc.tensor.transpose(
                                    ptv4[:, kt * 64:(kt + 1) * 64],
                                    vsb[hh * 64:(hh + 1) * 64,
                                        kt * 128:(kt + 1) * 128],
                                    ident[hh * 64:(hh + 1) * 64,
                                          hh * 64:(hh + 1) * 64])
                            k0 = (cc0 // 128)
                            dstv = vE[b][h_][:, k0 * 65:(k0 + 4) * 65] \
                                .rearrange("p (a t) -> p a t", t=65)
                            nc.vector.tensor_copy(
                                dstv[:, :, 0:64],
                                ptv4[:].rearrange("p (a t) -> p a t", a=4))
                            nc.vector.memset(
                                dstv[:, :, 64:65], 1.0)
                    else:
                        dst = qT[b][m] if nm == "q" else kT[b][m]
                        nc.scalar.copy(dst[:, cc0:cc0 + TC], pj[:])
            rope_piece(b, cchunk)

        def attention_qc(b, qc):
            # scores transposed: sT[k, q]; qc outer so o-quant batches
            # all 4 heads into [128, 256] pieces
            onat = sb_io.tile([128, 4 * 256], F32, tag="onat", bufs=1,
                              name="onat")
            rsum = sb_io.tile([128, 16], F32, tag="rsum", name="rsum")
            last = 4 * qc + 3
            for m in range(2):
                oTq2 = [ps_ot.tile([65, 512], F32, tag="ps_oT",
                                   name="ps_oT") for _ in range(2)]
                for kblk in range(4 * qc + 4):
                    qs0 = max(qc * 512, kblk * 128)
                    # pad diagonal blocks to >=256 wide (f32r/fp16
                    # matmuls need N>=256 for 1 cyc/row; padded cols
                    # are fully masked -> exp=0 -> PV adds zeros)
                    qs = min(qs0, (qc + 1) * 512 - 256)
                    w = (qc + 1) * 512 - qs
                    off = qs - qc * 512
                    pad = qs < qs0
                    # both heads of this m-tile back-to-back: their
                    # K=64 row groups (base 0 / 64) run concurrently
                    is_diag = kblk >= 4 * qc
                    scs = []
                    for hh in range(2):
                        p0 = hh * 64
                        sc = ps_sc.tile([128, 512], F32, tag="sc",
                                        name="sc")
                        scs.append(sc)
                        nc.tensor.matmul(
                            sc[:, 0:w],
                            kT[b][m][p0:p0 + 64,
                                     kblk * 128:(kblk + 1) * 128],
                            qT[b][m][p0:p0 + 64, qs:(qc + 1) * 512],
                            start=True, stop=True)
                    for hh in range(2):
                        h_ = m * 2 + hh
                        sc = scs[hh]
                        if is_diag:
                            if pad:
                                nc.vector.tensor_tensor(
                                    out=sc[:, 0:256], in0=sc[:, 0:256],
                                    in1=masksF[:, 0:256], op=ALU.add)
                            else:
                                nc.vector.tensor_tensor(
                                    out=sc[:, 0:128], in0=sc[:, 0:128],
                                    in1=masksF[:, 128:256], op=ALU.add)
                        pT = sb_pt.tile([128, 512], F16, tag="pT",
                                        name="pT")
                        nc.scalar.activation(pT[:, 0:w], sc[:, 0:w],
                                             ACTF.Exp, scale=0.125)
                        nc.tensor.matmul(
                            oTq2[hh][:, off:off + w],
                            vE[b][h_][:, kblk * 65:(kblk + 1) * 65],
                            pT[:, 0:w],
                            start=(kblk == 0),
                            stop=(kblk == last),
                            skip_group_check=(kblk == last and off != 0))
                # evacuate both heads: batch the 4 tt transposes
                for hh in range(2):
                    h_ = m * 2 + hh
                    osb = sb_io.tile([128, 512], F32, tag="osb",
                                     name="osb", bufs=1)
                    nc.vector.tensor_copy(osb[0:65, :], oTq2[hh][:])
                    ptn4 = ps_tr.tile([128, 260], F32, tag="ps_tr",
                                      name="ptn4")
                    for tt in range(4):
                        nc.tensor.transpose(
                            ptn4[:, tt * 65: tt * 65 + 65],
                            osb[0:65, tt * 128:(tt + 1) * 128],
                            ident[0:65, 0:65])
                    src4 = ptn4[:].rearrange("p (a t) -> p a t", t=65)
                    nc.vector.tensor_copy(
                        onat[:].rearrange("p (a t) -> p a t", a=4)
                        [:, :, h_ * 64:(h_ + 1) * 64],
                        src4[:, :, 0:64])
                    nc.vector.reciprocal(
                        rsum[:].rearrange("p (a t) -> p a t", t=4)
                        [:, :, h_:h_ + 1],
                        src4[:, :, 64:65])
            # quantize [128, 256] pieces (4 heads wide), fold 1/sum
            for tt in range(4):
                seg = onat[:, tt * 256:(tt + 1) * 256]
                amax = sb_tmp.tile([128, 64], F32, tag="am",
                                   name="am")[:, 0:16]
                nc.vector.tensor_reduce(
                    amax, seg.rearrange("p (b s) -> p b s", s=16),
                    axis=mybir.AxisListType.X, op=ALU.max,
                    apply_absolute_value=True)
                amc = sb_tmp.tile([128, 64], F32, tag="ac",
                                  name="ac")[:, 0:16]
                nc.vector.tensor_scalar_max(amc, amax, 1e-30)
                rcp = sb_tmp.tile([128, 64], F32, tag="rc",
                                  name="rc")[:, 0:16]
                nc.vector.reciprocal(rcp, amc)
                rs6 = sb_tmp.tile([128, 64], F32, tag="r6",
                                  name="r6")[:, 0:16]
                nc.vector.tensor_scalar_mul(rs6, rcp, 6.0)
                sct = sb_tmp.tile([128, 64], F32, tag="sc",
                                  name="sct")[:, 0:16]
                nc.vector.tensor_tensor(
                    out=sct.rearrange("p (h s) -> p h s", s=4),
                    in0=amax.rearrange("p (h s) -> p h s", s=4),
                    in1=rsum[:, tt * 4:(tt + 1) * 4].unsqueeze(2)
                    .broadcast_to([128, 4, 4]),
                    op=ALU.mult)
                nc.vector.tensor_scalar_mul(sct, sct, 1.0 / 6.0)
                oq = sb_io.tile([128, 256], F32, tag="oq", name="oq",
                                bufs=1)
                _quant(nc, sb_tmp, oq[:], seg, sct, rs6, 256)
                # transpose into oqT: cols h*64.. go to rows mm*128..
                tglob = qc * 4 + tt
                ptq2 = ps_tr.tile([128, 256], F32, tag="ps_tr",
                                  name="ptq2")
                for mm in range(2):
                    nc.tensor.transpose(
                        ptq2[:, mm * 128:(mm + 1) * 128],
                        oq[:, mm * 128:(mm + 1) * 128], ident[:])
                nc.vector.tensor_copy(
                    oqT[b][:].rearrange("p (a t) -> p a t", a=2)
                    [:, :, tglob * 128:(tglob + 1) * 128],
                    ptq2[:].rearrange("p (a t) -> p a t", a=2))

        def oproj_chunk(b, tch):
            t0 = b * S
            tc0 = tch * 512
            for mo in range(16):
                po = ps_po.tile([128, 512], F32, tag="po")
                for i in range(2):
                    nc.tensor.matmul(
                        po[:],
                        woT[:, i * HID + mo * 128:
                            i * HID + (mo + 1) * 128],
                        oqT[b][:, i * S + tc0: i * S + tc0 + 512],
                        start=(i == 0), stop=(i == 1))
                posb = sb_io.tile([128, 512], BF16, tag="posb",
                                  name="posb")
                if b == 0 and mo % 2 == 0:
                    nc.scalar.copy(posb[:], po[:])
                else:
                    nc.vector.tensor_copy(posb[:], po[:])
                nc.sync.dma_start(
                    out_d[mo * 128:(mo + 1) * 128,
                          t0 + tc0:t0 + tc0 + 512],
                    posb[:])

        # ---- batch 0 projections; x_prep(1)/wo interleaved ----
        wo_prep()
        for cc in range(NCH):
            proj_chunk(0, cc)
        # ---- b0 attention interleaved with b1 projections; b0's o_proj
        # is deferred into the tail (PE filler while b1 attention keeps
        # the ACT engine busy); b1 attention runs longest-qc first so the
        # final serial drain is the shortest qc ----
        for qc in range(4):
            attention_qc(0, qc)
            oproj_chunk(0, qc)
            proj_chunk(1, qc)
        for qc in range(4):
            attention_qc(1, qc)
            oproj_chunk(1, qc)

    nc.compile()
    return nc


def _np_quant(x):
    """Host fp4 fake-quant, bit-exact to the device implementation."""
    sh = x.shape
    xb = x.reshape(sh[:-1] + (sh[-1] // 16, 16)).astype(np.float32)
    amax = np.max(np.abs(xb), axis=-1, keepdims=True).astype(np.float32)
    amax_c = np.maximum(amax, np.float32(1e-30))
    rcp = (np.float32(1.0) / amax_c).astype(np.float32)
    rs6 = (rcp * np.float32(6.0)).astype(np.float32)
    scale = (amax * np.float32(1.0 / 6.0)).astype(np.float32)
    y = (xb * rs6).astype(np.float32)
    yi = y.view(np.int32)
    rem = yi & 0x3FFFFF
    inc = (rem > 0x200000).astype(np.int32) << 22
    h = ((yi & np.int32(-4194304)) + inc).view(np.float32)
    M32 = np.float32(MAGIC)
    low = ((y + M32).astype(np.float32) - M32).astype(np.float32)
    q = np.where(np.abs(y) > np.float32(2.0), h, low)
    return (q * scale).astype(np.float32).reshape(sh)


_HOST_CACHE = {}


def _host_tables():
    if _HOST_CACHE:
        return _HOST_CACHE
    D = HD
    inv = (1.0 / (10000.0 ** (np.arange(0, D, 2, dtype=np.float32)
                              / np.float32(D)))).astype(np.float32)
    fr = (np.arange(S, dtype=np.float32)[:, None] * inv[None, :]).astype(
        np.float32)
    cos = np.concatenate([np.cos(fr), np.cos(fr)], -1).astype(np.float32)
    sin = np.concatenate([np.sin(fr), np.sin(fr)], -1).astype(np.float32)
    cosT = np.zeros((128, T), np.float32)
    sinTs = np.zeros((128, T), np.float32)
    sgn = np.where(np.arange(D) < D // 2, np.float32(-1.0), np.float32(1.0))
    for bb in range(B):
        cosT[:, bb * S:(bb + 1) * S] = np.tile(cos.T, (2, 1))
        sinTs[:, bb * S:(bb + 1) * S] = np.tile((sin * sgn[None, :]).T,
                                                (2, 1))
    # mask table [128, 256], sT layout: col j (global q = qs + j), row k:
    # masked (NEG) iff (j - 128) < k.  cols 0-127: fully masked (used for
    # padded diagonal blocks); cols 128-255: the standard triangle.
    masks = np.zeros((128, 256), np.float32)
    for kk in range(128):
        masks[kk, :128 + kk] = NEG
    _HOST_CACHE.update(cosT=cosT, sinTs=sinTs, masks=masks)
    return _HOST_CACHE


_NC_CACHE = []


def make_in_maps(hidden_states, Wq, Wk, Wv, Wo):
    tabs = _host_tables()
    xf = hidden_states.reshape(T, HID)
    xq16 = np.ascontiguousarray(
        _np_quant(np.asarray(xf, np.float32)).T.reshape(16, 128, T)
        .astype(np.float16))
    wq_q = _np_quant(np.asarray(Wq, np.float32))
    wk_q = _np_quant(np.asarray(Wk, np.float32))
    wv_q = _np_quant(np.asarray(Wv, np.float32))
    wo_q = _np_quant(np.asarray(Wo, np.float32))
    import ml_dtypes
    bf16 = ml_dtypes.bfloat16
    in_maps = []
    for c in range(NCORES):
        sl = slice(c * OD, (c + 1) * OD)
        wqT = np.ascontiguousarray(
            wq_q[sl, :].T.reshape(16, 128, OD).astype(np.float16))
        wkT = np.ascontiguousarray(
            wk_q[sl, :].T.reshape(16, 128, OD).astype(np.float16))
        wvT = np.ascontiguousarray(
            wv_q[sl, :].T.reshape(16, 128, OD).astype(np.float16))
        woTc = np.ascontiguousarray(
            wo_q[:, sl].T.reshape(2, 128, HID).astype(bf16))
        in_maps.append(dict(
            xqT=xq16,
            wqT=wqT, wkT=wkT, wvT=wvT, woT=woTc,
            cosT=tabs['cosT'], sinTs=tabs['sinTs'], masks=tabs['masks'],
        ))
    return in_maps


def kernel(hidden_states, Wq, Wk, Wv, Wo):
    in_maps = make_in_maps(hidden_states, Wq, Wk, Wv, Wo)
    if not _NC_CACHE:
        _NC_CACHE.append(build())
    nc = _NC_CACHE[0]
    res = bass_utils.run_bass_kernel_spmd(nc, in_maps,
                                          core_ids=list(range(NCORES)))
    total = np.zeros((HID, T), np.float32)
    for r in res.results:
        total += np.asarray(r["partialT"], dtype=np.float32)
    return np.ascontiguousarray(total.T.reshape(B, S, HID))


if __name__ == "__main__":
    d = np.load('/root/problem/inputs.npz')
    out = kernel(d['hidden_states'], d['Wq'], d['Wk'], d['Wv'], d['Wo'])
    ref = np.load('/root/problem/ref_out.npy')
    rel2 = np.linalg.norm(out - ref) / np.linalg.norm(ref)
    print(f"relL2={rel2:.3e} absmax={np.abs(out - ref).max():.3e}")


# revision 28
# speedup vs baseline: 1.1270x; 1.0210x over previous
"""Trainium2 Bass kernel for AttentionWithFP4Projections.

Sharding: tensor-parallel over heads across 8 cores (4 heads each, both
batches). Each core computes q/k/v for its 256 output dims, full causal
attention for its heads, and a partial o_proj (its 256-dim slice of the
contraction); partials are summed on the host (no device collectives).

Numerics: FP4 fake-quant reproduced bit-exactly (magic-constant rounding
within fp32-ALU-exact ranges), split DVE (mults/select) + GpSimd (bit ops).
Matmul dtypes chosen for PE speed (1 cyc/row instead of fp32's 4):
  - q/k/v projections + scores: fp16 operands (quantized values rounded
    to fp16; ~5e-4 rel, fine through softmax),
  - PV (o accumulation): float32r (fp22) - o feeds the o-quant whose
    bucket decisions amplify pre-quant error, so keep it >= fp22,
  - o_proj: bf16 (post-quant, no amplification).
Softmax without max-subtraction (max scaled score ~5, no overflow);
normalization folded into the o-quant scale via a ones-column in V.
"""
import sys
import types
from contextlib import ExitStack

import numpy as np

# The NTFF profiling hook module is missing in this image; shim it so
# run_bass_kernel_spmd(trace=True) works (used by test.py, harmless here).
if 'antenv.axon_hooks' not in sys.modules:
    _m = types.ModuleType('antenv.axon_hooks')
    _m._hook = None
    _m.set_axon_ntff_profile_hook = lambda h: setattr(_m, '_hook', h)
    _m.get_axon_ntff_profile_hook = lambda: _m._hook
    sys.modules['antenv.axon_hooks'] = _m
    try:
        from trn_agent_boot.trn_boot import _ntff_profile_via_ctypes
        _m._hook = _ntff_profile_via_ctypes('/opt/axon/libaxon_pjrt.so')
    except Exception:
        pass

import concourse.mybir as mybir
import concourse.tile as tile
from concourse import bacc
from concourse import bass_utils
from concourse.masks import make_identity

# ---- custom fused DVE ops for the FP4 grid rounding ----------------------
# Extends the module-level registry (the documented add-an-op flow; the
# dve_ops.py file itself is read-only in this image).  Rows 17/18 are free
# (16 stock ops occupy 1..16; the byte-36 row field allows [1, 0x20)).
import concourse.dve_ops as _dvo
from concourse.dve_spec import (Spec as _Spec, Src0 as _Src0, Src1 as _Src1,
                                C0 as _C0, C1 as _C1, C2 as _C2,
                                Zero as _Zero, One as _One,
                                maxx as _maxx, minn as _minn, sq as _sq)


def _register_fp4_ops():
    import numpy as _np
    by_name = {o.name: o for o in _dvo.OPS}
    if "FP4_HI_ANT" in by_name:
        return by_name["FP4_HI_ANT"], by_name["FP4_LO_MERGE_ANT"]
    _ysq = _sq(_Src0)
    _i1 = _C0 < _ysq   # s0 = 2.5**2
    _i2 = _C1 < _ysq   # s1 = 3.5**2
    _i3 = _C2 < _ysq   # imm2 = 5**2
    # habs = 3*(|y|>2.5) + (|y|>3.5) + 2*(|y|>5), via (i1+i3)*2 + i2 + i1
    specB = _Spec(
        body=(_i1 + _i3) * (_One + _One) + _i2 + _i1,
        reference=lambda in0, s0, s1, imm2: (
            ((in0 * in0 > s0).astype(_np.float32) + (in0 * in0 > imm2)) * 2.0
            + (in0 * in0 > s1) + (in0 * in0 > s0)),
    )
    # g = clamp(magic_round(y), -2, 2) * (y*y <= 6.25) + in1(signed high)
    specA = _Spec(
        body=_minn(_maxx((_Src0 + _C0) - _C0, _C2), _Zero - _C2)
        * (_sq(_Src0) <= _C1) + _Src1,
        reference=lambda in0, in1, s0, s1, imm2: _np.clip(
            (in0 + _np.float32(s0)).astype(_np.float32) - _np.float32(s0),
            imm2, -imm2) * (in0 * in0 <= s1) + in1,
    )
    opB = _dvo.DveOp("FP4_HI_ANT", specB, subdim=False,
                     uops_sha={"v3": "176720cc7ee0a7f8",
                               "v4": "014accfcba4ba70e"})
    opA = _dvo.DveOp("FP4_LO_MERGE_ANT", specA, subdim=False,
                     uops_sha={"v3": "b57d557c01bd412c",
                               "v4": "780a6585d0fe9dbb"})
    _dvo.OPS.extend([opB, opA])
    _dvo._SUB_OPCODE_FOR_NAME[opB.name] = 17
    _dvo._SUB_OPCODE_FOR_NAME[opA.name] = 18
    _dvo.CUSTOM_DVE_SPECS[opB.name] = specB
    _dvo.CUSTOM_DVE_SPECS[opA.name] = specA
    return opB, opA


FP4_HI, FP4_LO_MERGE = _register_fp4_ops()

F32 = mybir.dt.float32
F32R = mybir.dt.float32r
F16 = mybir.dt.float16
BF16 = mybir.dt.bfloat16
I32 = mybir.dt.int32
ALU = mybir.AluOpType
ACTF = mybir.ActivationFunctionType

NCORES = 8
B, S, HID = 2, 2048, 2048
T = B * S                     # 4096 tokens
NH, HD = 32, 64               # heads, head dim
HPC = NH // NCORES            # 4 heads per core
OD = HPC * HD                 # 256 output dims per core
SPC = S // NCORES             # 256 tokens per batch per core
TC = 512                      # token-chunk width for projections
QW = 512                      # quantization sub-width (temp buffer size)
MAGIC = 6291456.0             # 1.5*2^22: +/- rounds fp32 to multiples of 0.5
NEG = -1.0e30


def _quant(nc, sb_tmp, out_ap, in_ap, scale_ap, rs6_ap, W, P=128):
    """FP4 fake-quant of in_ap [P, W] -> out_ap, given per-block scale and
    rs6 (=6/amax) [P, W//16].  Matches the jnp reference up to 1-ulp
    boundary/tie cases (reciprocal-based scale path, squared-bound
    compares).  Uses two fused custom DVE ops: 6 big passes total."""
    nb = W // 16
    y = sb_tmp.tile([128, QW], F32, tag="qt_y", name="qt_y")[:P, :W]
    nc.vector.tensor_tensor(
        out=y.rearrange("p (b s) -> p b s", s=16),
        in0=in_ap.rearrange("p (b s) -> p b s", s=16),
        in1=rs6_ap.unsqueeze(2).broadcast_to([P, nb, 16]),
        op=ALU.mult)
    h = sb_tmp.tile([128, QW], F32, tag="qt_h", name="qt_h")[:P, :W]
    nc.vector._custom_dve(FP4_HI, out=h, in0=y,
                          s0=6.25, s1=12.25, imm2=25.0)
    sbit = sb_tmp.tile([128, QW], I32, tag="qt_s", name="qt_s")[:P, :W]
    nc.vector.tensor_scalar(out=sbit, in0=in_ap.bitcast(I32),
                            scalar1=-2147483648, scalar2=None,
                            op0=ALU.bitwise_and)
    nc.vector.tensor_tensor(out=h.bitcast(I32), in0=h.bitcast(I32),
                            in1=sbit, op=ALU.bitwise_or)
    g = sb_tmp.tile([128, QW], F32, tag="qt_g", name="qt_g")[:P, :W]
    nc.vector._custom_dve(FP4_LO_MERGE, out=g, in0=y, in1=h,
                          s0=MAGIC, s1=6.25, imm2=-2.0)
    nc.vector.tensor_tensor(
        out=out_ap.rearrange("p (b s) -> p b s", s=16),
        in0=g.rearrange("p (b s) -> p b s", s=16),
        in1=scale_ap.unsqueeze(2).broadcast_to([P, nb, 16]),
        op=ALU.mult)


def _amax_scales(nc, sb_tmp, in_ap, W, P=128):
    """Returns (scale, rs6) [P, W//16] tiles for fp4 quant of in_ap."""
    nb = W // 16
    amax = sb_tmp.tile([128, 64], F32, tag="am", name="am")[:P, :nb]
    nc.vector.tensor_reduce(amax, in_ap.rearrange("p (b s) -> p b s", s=16),
                            axis=mybir.AxisListType.X, op=ALU.max,
                            apply_absolute_value=True)
    amc = sb_tmp.tile([128, 64], F32, tag="ac", name="ac")[:P, :nb]
    nc.vector.tensor_scalar_max(amc, amax, 1e-30)
    rcp = sb_tmp.tile([128, 64], F32, tag="rc", name="rc")[:P, :nb]
    nc.vector.reciprocal(rcp, amc)
    rs6 = sb_tmp.tile([128, 64], F32, tag="r6", name="r6")[:P, :nb]
    nc.vector.tensor_scalar_mul(rs6, rcp, 6.0)
    scale = sb_tmp.tile([128, 64], F32, tag="sc", name="sc")[:P, :nb]
    nc.vector.tensor_scalar_mul(scale, amax, 1.0 / 6.0)
    return scale, rs6, amax


def build():
    nc = bacc.Bacc("TRN2", target_bir_lowering=False, debug=False,
                   num_devices=NCORES)
    xq_d = nc.dram_tensor("xqT", [16, 128, T], F16,
                          kind="ExternalInput").ap()  # host-prequantized x^T
    wq_d = nc.dram_tensor("wqT", [16, 128, OD], F16,
                          kind="ExternalInput").ap()
    wk_d = nc.dram_tensor("wkT", [16, 128, OD], F16,
                          kind="ExternalInput").ap()
    wv_d = nc.dram_tensor("wvT", [16, 128, OD], F16,
                          kind="ExternalInput").ap()
    wo_d = nc.dram_tensor("woT", [2, 128, HID], BF16,
                          kind="ExternalInput").ap()
    cos_d = nc.dram_tensor("cosT", [128, T], F32, kind="ExternalInput").ap()
    sin_d = nc.dram_tensor("sinTs", [128, T], F32, kind="ExternalInput").ap()
    mask_d = nc.dram_tensor("masks", [128, 256], F32,
                            kind="ExternalInput").ap()
    out_d = nc.dram_tensor("partialT", [HID, T], BF16,
                           kind="ExternalOutput").ap()

    with tile.TileContext(nc) as tc, ExitStack() as ctx:
        sb_w = ctx.enter_context(tc.tile_pool(name="sb_w", bufs=1))
        sb_tmp = ctx.enter_context(tc.tile_pool(name="sb_tmp", bufs=1))
        sb_io = ctx.enter_context(tc.tile_pool(name="sb_io", bufs=2))
        sb_att = ctx.enter_context(tc.tile_pool(name="sb_att", bufs=1))
        sb_pt = ctx.enter_context(tc.tile_pool(name="sb_pt", bufs=2))
        # PSUM: 8 banks total.  Separate tags per stream so batch-b1
        # projections can overlap batch-b0 attention without pool-slot
        # serialization: pj(2, shared with o_proj) + sc(3) + oT(2) + tr(1) = 8.
        ps_pj = ctx.enter_context(
            tc.tile_pool(name="ps_pj", bufs=2, space="PSUM"))
        ps_sc = ctx.enter_context(
            tc.tile_pool(name="ps_sc", bufs=2, space="PSUM"))
        ps_po = ctx.enter_context(
            tc.tile_pool(name="ps_po", bufs=1, space="PSUM"))
        ps_ot = ctx.enter_context(
            tc.tile_pool(name="ps_ot", bufs=2, space="PSUM"))
        ps_tr = ctx.enter_context(
            tc.tile_pool(name="ps_tr", bufs=1, space="PSUM"))

        ident = sb_w.tile([128, 128], F32)
        make_identity(nc, ident[:])
        masksF = sb_w.tile([128, 256], F32)
        nc.sync.dma_start(masksF[:], mask_d)

        def quant_rows(dst_ap, src_ap, W):
            """quantize src [128, W] into dst, splitting into QW pieces."""
            for off in range(0, W, QW):
                w = min(QW, W - off)
                scale, rs6, _ = _amax_scales(nc, sb_tmp,
                                             src_ap[:, off:off + w], w)
                _quant(nc, sb_tmp, dst_ap[:, off:off + w],
                       src_ap[:, off:off + w], scale, rs6, w)


        # --------- weights: pre-quantized + transposed on host ---------
        wT = {}
        for nm, wd in (("q", wq_d), ("k", wk_d), ("v", wv_d)):
            wt = sb_w.tile([128, 16 * OD], F16, name=f"w{nm}T")
            wT[nm] = wt
            nc.sync.dma_start(wt[:].rearrange("p (a t) -> p a t", a=16),
                              wd.rearrange("a p t -> p a t"))
        woT = sb_w.tile([128, 2 * HID], BF16, name="woT")

        def wo_prep():
            for a in range(2):
                nc.sync.dma_start(woT[:, a * HID:(a + 1) * HID], wo_d[a])

        # persistent per-batch attention buffers (double-buffered across
        # batches so b1 projections overlap b0 attention)
        qT = {b: [sb_att.tile([128, S], F16, name=f"qT{b}{m}")
                  for m in range(2)] for b in range(B)}
        kT = {b: [sb_att.tile([128, S], F16, name=f"kT{b}{m}")
                  for m in range(2)] for b in range(B)}
        vE = {b: [sb_att.tile([128, 16 * 65], F16, name=f"vE{b}{h}")
                  for h in range(HPC)] for b in range(B)}
        oqT = {b: sb_att.tile([128, 2 * S], BF16, name=f"oqT{b}")
               for b in range(B)}

        NCH = S // TC  # chunks per batch (4)

        def rope_piece(b, pc):
            t0 = b * S
            for dst in (qT[b], kT[b]):
                for m in range(2):
                    c0 = pc * 512
                    cosT = sb_io.tile([128, 512], F32, tag="rope_c", bufs=1)
                    sinT = sb_io.tile([128, 512], F32, tag="rope_s", bufs=1)
                    nc.sync.dma_start(cosT[:],
                                      cos_d[:, t0 + c0:t0 + c0 + 512])
                    nc.sync.dma_start(sinT[:],
                                      sin_d[:, t0 + c0:t0 + c0 + 512])
                    sh = sb_io.tile([128, 512], F16, tag="rope_sh", bufs=1)
                    for hh in range(2):
                        p0 = hh * 64
                        nc.sync.dma_start(
                            sh[p0:p0 + 32, :],
                            dst[m][p0 + 32:p0 + 64, c0:c0 + 512])
                        nc.sync.dma_start(
                            sh[p0 + 32:p0 + 64, :],
                            dst[m][p0:p0 + 32, c0:c0 + 512])
                    tcos = sb_io.tile([128, 512], F32, tag="rope_tc", bufs=1)
                    shs = sb_io.tile([128, 512], F32, tag="rope_ss", bufs=1)
                    nc.vector.tensor_tensor(
                        out=tcos[:], in0=dst[m][:, c0:c0 + 512],
                        in1=cosT[:], op=ALU.mult)
                    nc.vector.tensor_tensor(out=shs[:], in0=sh[:],
                                            in1=sinT[:], op=ALU.mult)
                    nc.vector.tensor_tensor(
                        out=dst[m][:, c0:c0 + 512], in0=tcos[:],
                        in1=shs[:], op=ALU.add)

        def proj_chunk(b, cchunk):
            cc0 = cchunk * TC
            xqT = sb_pt.tile([128, 16 * TC], F16, tag="xqT", name="xqT")
            nc.sync.dma_start(
                xqT[:].rearrange("p (a t) -> p a t", a=16),
                xq_d[:, :, b * S + cc0: b * S + cc0 + TC]
                .rearrange("a p t -> p a t"))
            for nm in ("q", "k", "v"):
                for m in range(2):
                    pj = ps_pj.tile([128, TC], F32, tag="pj")
                    for i in range(16):
                        nc.tensor.matmul(
                            pj[:],
                            wT[nm][:, i * OD + m * 128:
                                   i * OD + (m + 1) * 128],
                            xqT[:, i * TC:(i + 1) * TC],
                            start=(i == 0), stop=(i == 15))
                    if nm == "v":
                        # to v-natural tiles with a ones column
                        vsb = sb_io.tile([128, TC], F32, tag="vsb")
                        nc.vector.tensor_copy(vsb[:], pj[:])
                        for hh in range(2):
                            h_ = m * 2 + hh
                            ptv4 = ps_tr.tile([128, 256], F32, tag="ps_tr")
                            for kt in range(TC // 128):
                                nc.tensor.transpose(
                                    ptv4[:, kt * 64:(kt + 1) * 64],
                                    vsb[hh * 64:(hh + 1) * 64,
                                        kt * 128:(kt + 1) * 128],
                                    ident[hh * 64:(hh + 1) * 64,
                                          hh * 64:(hh + 1) * 64])
                            k0 = (cc0 // 128)
                            dstv = vE[b][h_][:, k0 * 65:(k0 + 4) * 65] \
                                .rearrange("p (a t) -> p a t", t=65)
                            nc.vector.tensor_copy(
                                dstv[:, :, 0:64],
                                ptv4[:].rearrange("p (a t) -> p a t", a=4))
                            nc.vector.memset(
                                dstv[:, :, 64:65], 1.0)
                    else:
                        dst = qT[b][m] if nm == "q" else kT[b][m]
                        nc.scalar.copy(dst[:, cc0:cc0 + TC], pj[:])
            rope_piece(b, cchunk)

        def attention_qc(b, qc):
            # scores transposed: sT[k, q]; qc outer so o-quant batches
            # all 4 heads into [128, 256] pieces
            onat = sb_io.tile([128, 4 * 256], F32, tag="onat", bufs=1,
                              name="onat")
            rsum = sb_io.tile([128, 16], F32, tag="rsum", name="rsum")
            last = 4 * qc + 3
            for m in range(2):
                oTq2 = [ps_ot.tile([65, 512], F32, tag="ps_oT",
                                   name="ps_oT") for _ in range(2)]
                for kblk in range(4 * qc + 4):
                    qs = max(qc * 512, kblk * 128)
                    w = (qc + 1) * 512 - qs
                    off = qs - qc * 512
                    pad = False
                    # both heads of this m-tile back-to-back: their
                    # K=64 row groups (base 0 / 64) run concurrently
                    is_diag = kblk >= 4 * qc
                    scs = []
                    for hh in range(2):
                        p0 = hh * 64
                        sc = ps_sc.tile([128, 512], F32, tag="sc",
                                        name="sc")
                        scs.append(sc)
                        nc.tensor.matmul(
                            sc[:, 0:w],
                            kT[b][m][p0:p0 + 64,
                                     kblk * 128:(kblk + 1) * 128],
                            qT[b][m][p0:p0 + 64, qs:(qc + 1) * 512],
                            start=True, stop=True)
                    for hh in range(2):
                        h_ = m * 2 + hh
                        sc = scs[hh]
                        if is_diag:
                            nc.vector.tensor_tensor(
                                out=sc[:, 0:128], in0=sc[:, 0:128],
                                in1=masksF[:, 128:256], op=ALU.add)
                        pT = sb_pt.tile([128, 512], F16, tag="pT",
                                        name="pT")
                        nc.scalar.activation(pT[:, 0:w], sc[:, 0:w],
                                             ACTF.Exp, scale=0.125)
                        nc.tensor.matmul(
                            oTq2[hh][:, off:off + w],
                            vE[b][h_][:, kblk * 65:(kblk + 1) * 65],
                            pT[:, 0:w],
                            start=(kblk == 0),
                            stop=(kblk == last),
                            skip_group_check=(kblk == last and off != 0))
                # evacuate both heads: batch the 4 tt transposes
                for hh in range(2):
                    h_ = m * 2 + hh
                    osb = sb_io.tile([128, 512], F32, tag="osb",
                                     name="osb", bufs=1)
                    nc.vector.tensor_copy(osb[0:65, :], oTq2[hh][:])
                    ptn4 = ps_tr.tile([128, 260], F32, tag="ps_tr",
                                      name="ptn4")
                    for tt in range(4):
                        nc.tensor.transpose(
                            ptn4[:, tt * 65: tt * 65 + 65],
                            osb[0:65, tt * 128:(tt + 1) * 128],
                            ident[0:65, 0:65])
                    src4 = ptn4[:].rearrange("p (a t) -> p a t", t=65)
                    nc.vector.tensor_copy(
                        onat[:].rearrange("p (a t) -> p a t", a=4)
                        [:, :, h_ * 64:(h_ + 1) * 64],
                        src4[:, :, 0:64])
                    nc.vector.reciprocal(
                        rsum[:].rearrange("p (a t) -> p a t", t=4)
                        [:, :, h_:h_ + 1],
                        src4[:, :, 64:65])
            # quantize [128, 256] pieces (4 heads wide), fold 1/sum
            for tt in range(4):
                seg = onat[:, tt * 256:(tt + 1) * 256]
                amax = sb_tmp.tile([128, 64], F32, tag="am",
                                   name="am")[:, 0:16]
                nc.vector.tensor_reduce(
                    amax, seg.rearrange("p (b s) -> p b s", s=16),
                    axis=mybir.AxisListType.X, op=ALU.max,
                    apply_absolute_value=True)
                amc = sb_tmp.tile([128, 64], F32, tag="ac",
                                  name="ac")[:, 0:16]
                nc.vector.tensor_scalar_max(amc, amax, 1e-30)
                rcp = sb_tmp.tile([128, 64], F32, tag="rc",
                                  name="rc")[:, 0:16]
                nc.vector.reciprocal(rcp, amc)
                rs6 = sb_tmp.tile([128, 64], F32, tag="r6",
                                  name="r6")[:, 0:16]
                nc.vector.tensor_scalar_mul(rs6, rcp, 6.0)
                sct = sb_tmp.tile([128, 64], F32, tag="sc",
                                  name="sct")[:, 0:16]
                nc.vector.tensor_tensor(
                    out=sct.rearrange("p (h s) -> p h s", s=4),
                    in0=amax.rearrange("p (h s) -> p h s", s=4),
                    in1=rsum[:, tt * 4:(tt + 1) * 4].unsqueeze(2)
                    .broadcast_to([128, 4, 4]),
                    op=ALU.mult)
                nc.vector.tensor_scalar_mul(sct, sct, 1.0 / 6.0)
                oq = sb_io.tile([128, 256], F32, tag="oq", name="oq",
                                bufs=1)
                _quant(nc, sb_tmp, oq[:], seg, sct, rs6, 256)
                # transpose into oqT: cols h*64.. go to rows mm*128..
                tglob = qc * 4 + tt
                ptq2 = ps_tr.tile([128, 256], F32, tag="ps_tr",
                                  name="ptq2")
                for mm in range(2):
                    nc.tensor.transpose(
                        ptq2[:, mm * 128:(mm + 1) * 128],
                        oq[:, mm * 128:(mm + 1) * 128], ident[:])
                nc.vector.tensor_copy(
                    oqT[b][:].rearrange("p (a t) -> p a t", a=2)
                    [:, :, tglob * 128:(tglob + 1) * 128],
                    ptq2[:].rearrange("p (a t) -> p a t", a=2))

        def oproj_chunk(b, tch):
            t0 = b * S
            tc0 = tch * 512
            for mo in range(16):
                po = ps_po.tile([128, 512], F32, tag="po")
                for i in range(2):
                    nc.tensor.matmul(
                        po[:],
                        woT[:, i * HID + mo * 128:
                            i * HID + (mo + 1) * 128],
                        oqT[b][:, i * S + tc0: i * S + tc0 + 512],
                        start=(i == 0), stop=(i == 1))
                posb = sb_io.tile([128, 512], BF16, tag="posb",
                                  name="posb")
                if b == 0 and mo % 2 == 0:
                    nc.scalar.copy(posb[:], po[:])
                else:
                    nc.vector.tensor_copy(posb[:], po[:])
                nc.sync.dma_start(
                    out_d[mo * 128:(mo + 1) * 128,
                          t0 + tc0:t0 + tc0 + 512],
                    posb[:])

        # ---- batch 0 projections; x_prep(1)/wo interleaved ----
        wo_prep()
        for cc in range(NCH):
            proj_chunk(0, cc)
        # ---- b0 attention interleaved with b1 projections; b0's o_proj
        # is deferred into the tail (PE filler while b1 attention keeps
        # the ACT engine busy); b1 attention runs longest-qc first so the
        # final serial drain is the shortest qc ----
        for qc in range(4):
            attention_qc(0, qc)
            oproj_chunk(0, qc)
            proj_chunk(1, qc)
        for qc in range(4):
            attention_qc(1, qc)
            oproj_chunk(1, qc)

    nc.compile()
    return nc


def _np_quant(x):
    """Host fp4 fake-quant, bit-exact to the device implementation."""
    sh = x.shape
    xb = x.reshape(sh[:-1] + (sh[-1] // 16, 16)).astype(np.float32)
    amax = np.max(np.abs(xb), axis=-1, keepdims=True).astype(np.float32)
    amax_c = np.maximum(amax, np.float32(1e-30))
    rcp = (np.float32(1.0) / amax_c).astype(np.float32)
    rs6 = (rcp * np.float32(6.0)).astype(np.float32)
    scale = (amax * np.float32(1.0 / 6.0)).astype(np.float32)
    y = (xb * rs6).astype(np.float32)
    yi = y.view(np.int32)
    rem = yi & 0x3FFFFF
    inc = (rem > 0x200000).astype(np.int32) << 22
    h = ((yi & np.int32(-4194304)) + inc).view(np.float32)
    M32 = np.float32(MAGIC)
    low = ((y + M32).astype(np.float32) - M32).astype(np.float32)
    q = np.where(np.abs(y) > np.float32(2.0), h, low)
    return (q * scale).astype(np.float32).reshape(sh)


_HOST_CACHE = {}


def _host_tables():
    if _HOST_CACHE:
        return _HOST_CACHE
    D = HD
    inv = (1.0 / (10000.0 ** (np.arange(0, D, 2, dtype=np.float32)
                              / np.float32(D)))).astype(np.float32)
    fr = (np.arange(S, dtype=np.float32)[:, None] * inv[None, :]).astype(
        np.float32)
    cos = np.concatenate([np.cos(fr), np.cos(fr)], -1).astype(np.float32)
    sin = np.concatenate([np.sin(fr), np.sin(fr)], -1).astype(np.float32)
    cosT = np.zeros((128, T), np.float32)
    sinTs = np.zeros((128, T), np.float32)
    sgn = np.where(np.arange(D) < D // 2, np.float32(-1.0), np.float32(1.0))
    for bb in range(B):
        cosT[:, bb * S:(bb + 1) * S] = np.tile(cos.T, (2, 1))
        sinTs[:, bb * S:(bb + 1) * S] = np.tile((sin * sgn[None, :]).T,
                                                (2, 1))
    # mask table [128, 256], sT layout: col j (global q = qs + j), row k:
    # masked (NEG) iff (j - 128) < k.  cols 0-127: fully masked (used for
    # padded diagonal blocks); cols 128-255: the standard triangle.
    masks = np.zeros((128, 256), np.float32)
    for kk in range(128):
        masks[kk, :128 + kk] = NEG
    _HOST_CACHE.update(cosT=cosT, sinTs=sinTs, masks=masks)
    return _HOST_CACHE


_NC_CACHE = []


def make_in_maps(hidden_states, Wq, Wk, Wv, Wo):
    tabs = _host_tables()
    xf = hidden_states.reshape(T, HID)
    xq16 = np.ascontiguousarray(
        _np_quant(np.asarray(xf, np.float32)).T.reshape(16, 128, T)
        .astype(np.float16))
    wq_q = _np_quant(np.asarray(Wq, np.float32))
    wk_q = _np_quant(np.asarray(Wk, np.float32))
    wv_q = _np_quant(np.asarray(Wv, np.float32))
    wo_q = _np_quant(np.asarray(Wo, np.float32))
    import ml_dtypes
    bf16 = ml_dtypes.bfloat16
    in_maps = []
    for c in range(NCORES):
        sl = slice(c * OD, (c + 1) * OD)
        wqT = np.ascontiguousarray(
            wq_q[sl, :].T.reshape(16, 128, OD).astype(np.float16))
        wkT = np.ascontiguousarray(
            wk_q[sl, :].T.reshape(16, 128, OD).astype(np.float16))
        wvT = np.ascontiguousarray(
            wv_q[sl, :].T.reshape(16, 128, OD).astype(np.float16))
        woTc = np.ascontiguousarray(
            wo_q[:, sl].T.reshape(2, 128, HID).astype(bf16))
        in_maps.append(dict(
            xqT=xq16,
            wqT=wqT, wkT=wkT, wvT=wvT, woT=woTc,
            cosT=tabs['cosT'], sinTs=tabs['sinTs'], masks=tabs['masks'],
        ))
    return in_maps


def kernel(hidden_states, Wq, Wk, Wv, Wo):
    in_maps = make_in_maps(hidden_states, Wq, Wk, Wv, Wo)
    if not _NC_CACHE:
        _NC_CACHE.append(build())
    nc = _NC_CACHE[0]
    res = bass_utils.run_bass_kernel_spmd(nc, in_maps,
                                          core_ids=list(range(NCORES)))
    total = np.zeros((HID, T), np.float32)
    for r in res.results:
        total += np.asarray(r["partialT"], dtype=np.float32)
    return np.ascontiguousarray(total.T.reshape(B, S, HID))


if __name__ == "__main__":
    d = np.load('/root/problem/inputs.npz')
    out = kernel(d['hidden_states'], d['Wq'], d['Wk'], d['Wv'], d['Wo'])
    ref = np.load('/root/problem/ref_out.npy')
    rel2 = np.linalg.norm(out - ref) / np.linalg.norm(ref)
    print(f"relL2={rel2:.3e} absmax={np.abs(out - ref).max():.3e}")


# revision 29
# speedup vs baseline: 1.1300x; 1.0026x over previous
"""Trainium2 Bass kernel for AttentionWithFP4Projections.

Sharding: tensor-parallel over heads across 8 cores (4 heads each, both
batches). Each core computes q/k/v for its 256 output dims, full causal
attention for its heads, and a partial o_proj (its 256-dim slice of the
contraction); partials are summed on the host (no device collectives).

Numerics: FP4 fake-quant reproduced bit-exactly (magic-constant rounding
within fp32-ALU-exact ranges), split DVE (mults/select) + GpSimd (bit ops).
Matmul dtypes chosen for PE speed (1 cyc/row instead of fp32's 4):
  - q/k/v projections + scores: fp16 operands (quantized values rounded
    to fp16; ~5e-4 rel, fine through softmax),
  - PV (o accumulation): float32r (fp22) - o feeds the o-quant whose
    bucket decisions amplify pre-quant error, so keep it >= fp22,
  - o_proj: bf16 (post-quant, no amplification).
Softmax without max-subtraction (max scaled score ~5, no overflow);
normalization folded into the o-quant scale via a ones-column in V.
"""
import sys
import types
from contextlib import ExitStack

import numpy as np

# The NTFF profiling hook module is missing in this image; shim it so
# run_bass_kernel_spmd(trace=True) works (used by test.py, harmless here).
if 'antenv.axon_hooks' not in sys.modules:
    _m = types.ModuleType('antenv.axon_hooks')
    _m._hook = None
    _m.set_axon_ntff_profile_hook = lambda h: setattr(_m, '_hook', h)
    _m.get_axon_ntff_profile_hook = lambda: _m._hook
    sys.modules['antenv.axon_hooks'] = _m
    try:
        from trn_agent_boot.trn_boot import _ntff_profile_via_ctypes
        _m._hook = _ntff_profile_via_ctypes('/opt/axon/libaxon_pjrt.so')
    except Exception:
        pass

import concourse.mybir as mybir
import concourse.tile as tile
from concourse import bacc
from concourse import bass_utils
from concourse.masks import make_identity

# ---- custom fused DVE ops for the FP4 grid rounding ----------------------
# Extends the module-level registry (the documented add-an-op flow; the
# dve_ops.py file itself is read-only in this image).  Rows 17/18 are free
# (16 stock ops occupy 1..16; the byte-36 row field allows [1, 0x20)).
import concourse.dve_ops as _dvo
from concourse.dve_spec import (Spec as _Spec, Src0 as _Src0, Src1 as _Src1,
                                C0 as _C0, C1 as _C1, C2 as _C2,
                                Zero as _Zero, One as _One,
                                maxx as _maxx, minn as _minn, sq as _sq)


def _register_fp4_ops():
    import numpy as _np
    by_name = {o.name: o for o in _dvo.OPS}
    if "FP4_HI_ANT" in by_name:
        return by_name["FP4_HI_ANT"], by_name["FP4_LO_MERGE_ANT"]
    _ysq = _sq(_Src0)
    _i1 = _C0 < _ysq   # s0 = 2.5**2
    _i2 = _C1 < _ysq   # s1 = 3.5**2
    _i3 = _C2 < _ysq   # imm2 = 5**2
    # habs = 3*(|y|>2.5) + (|y|>3.5) + 2*(|y|>5), via (i1+i3)*2 + i2 + i1
    specB = _Spec(
        body=(_i1 + _i3) * (_One + _One) + _i2 + _i1,
        reference=lambda in0, s0, s1, imm2: (
            ((in0 * in0 > s0).astype(_np.float32) + (in0 * in0 > imm2)) * 2.0
            + (in0 * in0 > s1) + (in0 * in0 > s0)),
    )
    # g = clamp(magic_round(y), -2, 2) * (y*y <= 6.25) + in1(signed high)
    specA = _Spec(
        body=_minn(_maxx((_Src0 + _C0) - _C0, _C2), _Zero - _C2)
        * (_sq(_Src0) <= _C1) + _Src1,
        reference=lambda in0, in1, s0, s1, imm2: _np.clip(
            (in0 + _np.float32(s0)).astype(_np.float32) - _np.float32(s0),
            imm2, -imm2) * (in0 * in0 <= s1) + in1,
    )
    opB = _dvo.DveOp("FP4_HI_ANT", specB, subdim=False,
                     uops_sha={"v3": "176720cc7ee0a7f8",
                               "v4": "014accfcba4ba70e"})
    opA = _dvo.DveOp("FP4_LO_MERGE_ANT", specA, subdim=False,
                     uops_sha={"v3": "b57d557c01bd412c",
                               "v4": "780a6585d0fe9dbb"})
    _dvo.OPS.extend([opB, opA])
    _dvo._SUB_OPCODE_FOR_NAME[opB.name] = 17
    _dvo._SUB_OPCODE_FOR_NAME[opA.name] = 18
    _dvo.CUSTOM_DVE_SPECS[opB.name] = specB
    _dvo.CUSTOM_DVE_SPECS[opA.name] = specA
    return opB, opA


FP4_HI, FP4_LO_MERGE = _register_fp4_ops()

F32 = mybir.dt.float32
F32R = mybir.dt.float32r
F16 = mybir.dt.float16
BF16 = mybir.dt.bfloat16
I32 = mybir.dt.int32
ALU = mybir.AluOpType
ACTF = mybir.ActivationFunctionType

NCORES = 8
B, S, HID = 2, 2048, 2048
T = B * S                     # 4096 tokens
NH, HD = 32, 64               # heads, head dim
HPC = NH // NCORES            # 4 heads per core
OD = HPC * HD                 # 256 output dims per core
SPC = S // NCORES             # 256 tokens per batch per core
TC = 512                      # token-chunk width for projections
QW = 512                      # quantization sub-width (temp buffer size)
MAGIC = 6291456.0             # 1.5*2^22: +/- rounds fp32 to multiples of 0.5
NEG = -1.0e30


def _quant(nc, sb_tmp, out_ap, in_ap, scale_ap, rs6_ap, W, P=128):
    """FP4 fake-quant of in_ap [P, W] -> out_ap, given per-block scale and
    rs6 (=6/amax) [P, W//16].  Matches the jnp reference up to 1-ulp
    boundary/tie cases (reciprocal-based scale path, squared-bound
    compares).  Uses two fused custom DVE ops: 6 big passes total."""
    nb = W // 16
    y = sb_tmp.tile([128, QW], F32, tag="qt_y", name="qt_y")[:P, :W]
    nc.vector.tensor_tensor(
        out=y.rearrange("p (b s) -> p b s", s=16),
        in0=in_ap.rearrange("p (b s) -> p b s", s=16),
        in1=rs6_ap.unsqueeze(2).broadcast_to([P, nb, 16]),
        op=ALU.mult)
    h = sb_tmp.tile([128, QW], F32, tag="qt_h", name="qt_h")[:P, :W]
    nc.vector._custom_dve(FP4_HI, out=h, in0=y,
                          s0=6.25, s1=12.25, imm2=25.0)
    sbit = sb_tmp.tile([128, QW], I32, tag="qt_s", name="qt_s")[:P, :W]
    nc.vector.tensor_scalar(out=sbit, in0=in_ap.bitcast(I32),
                            scalar1=-2147483648, scalar2=None,
                            op0=ALU.bitwise_and)
    nc.vector.tensor_tensor(out=h.bitcast(I32), in0=h.bitcast(I32),
                            in1=sbit, op=ALU.bitwise_or)
    g = sb_tmp.tile([128, QW], F32, tag="qt_g", name="qt_g")[:P, :W]
    nc.vector._custom_dve(FP4_LO_MERGE, out=g, in0=y, in1=h,
                          s0=MAGIC, s1=6.25, imm2=-2.0)
    nc.vector.tensor_tensor(
        out=out_ap.rearrange("p (b s) -> p b s", s=16),
        in0=g.rearrange("p (b s) -> p b s", s=16),
        in1=scale_ap.unsqueeze(2).broadcast_to([P, nb, 16]),
        op=ALU.mult)


def _amax_scales(nc, sb_tmp, in_ap, W, P=128):
    """Returns (scale, rs6) [P, W//16] tiles for fp4 quant of in_ap."""
    nb = W // 16
    amax = sb_tmp.tile([128, 64], F32, tag="am", name="am")[:P, :nb]
    nc.vector.tensor_reduce(amax, in_ap.rearrange("p (b s) -> p b s", s=16),
                            axis=mybir.AxisListType.X, op=ALU.max,
                            apply_absolute_value=True)
    amc = sb_tmp.tile([128, 64], F32, tag="ac", name="ac")[:P, :nb]
    nc.vector.tensor_scalar_max(amc, amax, 1e-30)
    rcp = sb_tmp.tile([128, 64], F32, tag="rc", name="rc")[:P, :nb]
    nc.vector.reciprocal(rcp, amc)
    rs6 = sb_tmp.tile([128, 64], F32, tag="r6", name="r6")[:P, :nb]
    nc.vector.tensor_scalar_mul(rs6, rcp, 6.0)
    scale = sb_tmp.tile([128, 64], F32, tag="sc", name="sc")[:P, :nb]
    nc.vector.tensor_scalar_mul(scale, amax, 1.0 / 6.0)
    return scale, rs6, amax


def build():
    nc = bacc.Bacc("TRN2", target_bir_lowering=False, debug=False,
                   num_devices=NCORES)
    xq_d = nc.dram_tensor("xqT", [16, 128, T], F16,
                          kind="ExternalInput").ap()  # host-prequantized x^T
    wq_d = nc.dram_tensor("wqT", [16, 128, OD], F16,
                          kind="ExternalInput").ap()
    wk_d = nc.dram_tensor("wkT", [16, 128, OD], F16,
                          kind="ExternalInput").ap()
    wv_d = nc.dram_tensor("wvT", [16, 128, OD], F16,
                          kind="ExternalInput").ap()
    wo_d = nc.dram_tensor("woT", [2, 128, HID], BF16,
                          kind="ExternalInput").ap()
    cos_d = nc.dram_tensor("cosT", [128, T], F32, kind="ExternalInput").ap()
    sin_d = nc.dram_tensor("sinTs", [128, T], F32, kind="ExternalInput").ap()
    mask_d = nc.dram_tensor("masks", [128, 256], F32,
                            kind="ExternalInput").ap()
    out_d = nc.dram_tensor("partialT", [HID, T], BF16,
                           kind="ExternalOutput").ap()

    with tile.TileContext(nc) as tc, ExitStack() as ctx:
        sb_w = ctx.enter_context(tc.tile_pool(name="sb_w", bufs=1))
        sb_tmp = ctx.enter_context(tc.tile_pool(name="sb_tmp", bufs=1))
        sb_io = ctx.enter_context(tc.tile_pool(name="sb_io", bufs=2))
        sb_att = ctx.enter_context(tc.tile_pool(name="sb_att", bufs=1))
        sb_pt = ctx.enter_context(tc.tile_pool(name="sb_pt", bufs=2))
        # PSUM: 8 banks total.  Separate tags per stream so batch-b1
        # projections can overlap batch-b0 attention without pool-slot
        # serialization: pj(2, shared with o_proj) + sc(3) + oT(2) + tr(1) = 8.
        ps_pj = ctx.enter_context(
            tc.tile_pool(name="ps_pj", bufs=2, space="PSUM"))
        ps_sc = ctx.enter_context(
            tc.tile_pool(name="ps_sc", bufs=2, space="PSUM"))
        ps_po = ctx.enter_context(
            tc.tile_pool(name="ps_po", bufs=1, space="PSUM"))
        ps_ot = ctx.enter_context(
            tc.tile_pool(name="ps_ot", bufs=2, space="PSUM"))
        ps_tr = ctx.enter_context(
            tc.tile_pool(name="ps_tr", bufs=1, space="PSUM"))

        ident = sb_w.tile([128, 128], F32)
        make_identity(nc, ident[:])
        masksF = sb_w.tile([128, 256], F32)
        nc.sync.dma_start(masksF[:], mask_d)

        def quant_rows(dst_ap, src_ap, W):
            """quantize src [128, W] into dst, splitting into QW pieces."""
            for off in range(0, W, QW):
                w = min(QW, W - off)
                scale, rs6, _ = _amax_scales(nc, sb_tmp,
                                             src_ap[:, off:off + w], w)
                _quant(nc, sb_tmp, dst_ap[:, off:off + w],
                       src_ap[:, off:off + w], scale, rs6, w)


        # --------- weights: pre-quantized + transposed on host ---------
        wT = {}
        for nm, wd in (("q", wq_d), ("k", wk_d), ("v", wv_d)):
            wt = sb_w.tile([128, 16 * OD], F16, name=f"w{nm}T")
            wT[nm] = wt
            nc.sync.dma_start(wt[:].rearrange("p (a t) -> p a t", a=16),
                              wd.rearrange("a p t -> p a t"))
        woT = sb_w.tile([128, 2 * HID], BF16, name="woT")

        def wo_prep():
            for a in range(2):
                nc.sync.dma_start(woT[:, a * HID:(a + 1) * HID], wo_d[a])

        # persistent per-batch attention buffers (double-buffered across
        # batches so b1 projections overlap b0 attention)
        qT = {b: [sb_att.tile([128, S], F16, name=f"qT{b}{m}")
                  for m in range(2)] for b in range(B)}
        kT = {b: [sb_att.tile([128, S], F16, name=f"kT{b}{m}")
                  for m in range(2)] for b in range(B)}
        vE = {b: [sb_att.tile([128, 16 * 65], F16, name=f"vE{b}{h}")
                  for h in range(HPC)] for b in range(B)}
        oqT = {b: sb_att.tile([128, 2 * S], BF16, name=f"oqT{b}")
               for b in range(B)}

        NCH = S // TC  # chunks per batch (4)

        def rope_piece(b, pc):
            t0 = b * S
            for dst in (qT[b], kT[b]):
                for m in range(2):
                    c0 = pc * 512
                    cosT = sb_io.tile([128, 512], F32, tag="rope_c", bufs=1)
                    sinT = sb_io.tile([128, 512], F32, tag="rope_s", bufs=1)
                    nc.sync.dma_start(cosT[:],
                                      cos_d[:, t0 + c0:t0 + c0 + 512])
                    nc.sync.dma_start(sinT[:],
                                      sin_d[:, t0 + c0:t0 + c0 + 512])
                    sh = sb_io.tile([128, 512], F16, tag="rope_sh", bufs=1)
                    for hh in range(2):
                        p0 = hh * 64
                        nc.sync.dma_start(
                            sh[p0:p0 + 32, :],
                            dst[m][p0 + 32:p0 + 64, c0:c0 + 512])
                        nc.sync.dma_start(
                            sh[p0 + 32:p0 + 64, :],
                            dst[m][p0:p0 + 32, c0:c0 + 512])
                    tcos = sb_io.tile([128, 512], F32, tag="rope_tc", bufs=1)
                    shs = sb_io.tile([128, 512], F32, tag="rope_ss", bufs=1)
                    nc.vector.tensor_tensor(
                        out=tcos[:], in0=dst[m][:, c0:c0 + 512],
                        in1=cosT[:], op=ALU.mult)
                    nc.vector.tensor_tensor(out=shs[:], in0=sh[:],
                                            in1=sinT[:], op=ALU.mult)
                    nc.vector.tensor_tensor(
                        out=dst[m][:, c0:c0 + 512], in0=tcos[:],
                        in1=shs[:], op=ALU.add)

        def proj_chunk(b, cchunk):
            cc0 = cchunk * TC
            xqT = sb_pt.tile([128, 16 * TC], F16, tag="xqT", name="xqT")
            nc.sync.dma_start(
                xqT[:].rearrange("p (a t) -> p a t", a=16),
                xq_d[:, :, b * S + cc0: b * S + cc0 + TC]
                .rearrange("a p t -> p a t"))
            for nm in ("q", "k", "v"):
                for m in range(2):
                    pj = ps_pj.tile([128, TC], F32, tag="pj")
                    for i in range(16):
                        nc.tensor.matmul(
                            pj[:],
                            wT[nm][:, i * OD + m * 128:
                                   i * OD + (m + 1) * 128],
                            xqT[:, i * TC:(i + 1) * TC],
                            start=(i == 0), stop=(i == 15))
                    if nm == "v":
                        # to v-natural tiles with a ones column
                        vsb = sb_io.tile([128, TC], F32, tag="vsb")
                        nc.vector.tensor_copy(vsb[:], pj[:])
                        for hh in range(2):
                            h_ = m * 2 + hh
                            ptv4 = ps_tr.tile([128, 256], F32, tag="ps_tr")
                            for kt in range(TC // 128):
                                nc.tensor.transpose(
                                    ptv4[:, kt * 64:(kt + 1) * 64],
                                    vsb[hh * 64:(hh + 1) * 64,
                                        kt * 128:(kt + 1) * 128],
                                    ident[hh * 64:(hh + 1) * 64,
                                          hh * 64:(hh + 1) * 64])
                            k0 = (cc0 // 128)
                            dstv = vE[b][h_][:, k0 * 65:(k0 + 4) * 65] \
                                .rearrange("p (a t) -> p a t", t=65)
                            nc.vector.tensor_copy(
                                dstv[:, :, 0:64],
                                ptv4[:].rearrange("p (a t) -> p a t", a=4))
                            nc.vector.memset(
                                dstv[:, :, 64:65], 1.0)
                    else:
                        dst = qT[b][m] if nm == "q" else kT[b][m]
                        nc.scalar.copy(dst[:, cc0:cc0 + TC], pj[:])
            rope_piece(b, cchunk)

        def attention_qc(b, qc):
            # scores transposed: sT[k, q]; qc outer so o-quant batches
            # all 4 heads into [128, 256] pieces
            onat = sb_io.tile([128, 4 * 256], F32, tag="onat", bufs=1,
                              name="onat")
            rsum = sb_io.tile([128, 16], F32, tag="rsum", name="rsum")
            last = 4 * qc + 3
            for m in range(2):
                oTq2 = [ps_ot.tile([65, 512], F32, tag="ps_oT",
                                   name="ps_oT") for _ in range(2)]
                for kblk in range(4 * qc + 4):
                    qs = max(qc * 512, kblk * 128)
                    w = (qc + 1) * 512 - qs
                    off = qs - qc * 512
                    pad = False
                    # both heads of this m-tile back-to-back: their
                    # K=64 row groups (base 0 / 64) run concurrently
                    is_diag = kblk >= 4 * qc
                    scs = []
                    for hh in range(2):
                        p0 = hh * 64
                        sc = ps_sc.tile([128, 512], F32, tag="sc",
                                        name="sc")
                        scs.append(sc)
                        nc.tensor.matmul(
                            sc[:, 0:w],
                            kT[b][m][p0:p0 + 64,
                                     kblk * 128:(kblk + 1) * 128],
                            qT[b][m][p0:p0 + 64, qs:(qc + 1) * 512],
                            start=True, stop=True)
                    for hh in range(2):
                        h_ = m * 2 + hh
                        sc = scs[hh]
                        if is_diag:
                            nc.vector.tensor_tensor(
                                out=sc[:, 0:128], in0=sc[:, 0:128],
                                in1=masksF[:, 128:256], op=ALU.add)
                        pT = sb_pt.tile([128, 512], F16, tag="pT",
                                        name="pT")
                        nc.scalar.activation(pT[:, 0:w], sc[:, 0:w],
                                             ACTF.Exp, scale=0.125)
                        nc.tensor.matmul(
                            oTq2[hh][:, off:off + w],
                            vE[b][h_][:, kblk * 65:(kblk + 1) * 65],
                            pT[:, 0:w],
                            start=(kblk == 0),
                            stop=(kblk == last),
                            skip_group_check=(kblk == last and off != 0))
                # evacuate both heads: batch the 4 tt transposes
                for hh in range(2):
                    h_ = m * 2 + hh
                    osb = sb_io.tile([128, 512], F32, tag="osb",
                                     name="osb", bufs=1)
                    nc.vector.tensor_copy(osb[0:65, :], oTq2[hh][:])
                    ptn4 = ps_tr.tile([128, 260], F32, tag="ps_tr",
                                      name="ptn4")
                    for tt in range(4):
                        nc.tensor.transpose(
                            ptn4[:, tt * 65: tt * 65 + 65],
                            osb[0:65, tt * 128:(tt + 1) * 128],
                            ident[0:65, 0:65])
                    src4 = ptn4[:].rearrange("p (a t) -> p a t", t=65)
                    nc.vector.tensor_copy(
                        onat[:].rearrange("p (a t) -> p a t", a=4)
                        [:, :, h_ * 64:(h_ + 1) * 64],
                        src4[:, :, 0:64])
                    nc.vector.reciprocal(
                        rsum[:].rearrange("p (a t) -> p a t", t=4)
                        [:, :, h_:h_ + 1],
                        src4[:, :, 64:65])
            # quantize [128, 256] pieces (4 heads wide), fold 1/sum
            for tt in range(4):
                seg = onat[:, tt * 256:(tt + 1) * 256]
                amax = sb_tmp.tile([128, 64], F32, tag="am",
                                   name="am")[:, 0:16]
                nc.vector.tensor_reduce(
                    amax, seg.rearrange("p (b s) -> p b s", s=16),
                    axis=mybir.AxisListType.X, op=ALU.max,
                    apply_absolute_value=True)
                amc = sb_tmp.tile([128, 64], F32, tag="ac",
                                  name="ac")[:, 0:16]
                nc.vector.tensor_scalar_max(amc, amax, 1e-30)
                rcp = sb_tmp.tile([128, 64], F32, tag="rc",
                                  name="rc")[:, 0:16]
                nc.vector.reciprocal(rcp, amc)
                rs6 = sb_tmp.tile([128, 64], F32, tag="r6",
                                  name="r6")[:, 0:16]
                nc.vector.tensor_scalar_mul(rs6, rcp, 6.0)
                sct = sb_tmp.tile([128, 64], F32, tag="sc",
                                  name="sct")[:, 0:16]
                nc.vector.tensor_tensor(
                    out=sct.rearrange("p (h s) -> p h s", s=4),
                    in0=amax.rearrange("p (h s) -> p h s", s=4),
                    in1=rsum[:, tt * 4:(tt + 1) * 4].unsqueeze(2)
                    .broadcast_to([128, 4, 4]),
                    op=ALU.mult)
                nc.vector.tensor_scalar_mul(sct, sct, 1.0 / 6.0)
                oq = sb_io.tile([128, 256], F32, tag="oq", name="oq",
                                bufs=1)
                _quant(nc, sb_tmp, oq[:], seg, sct, rs6, 256)
                # transpose into oqT: cols h*64.. go to rows mm*128..
                tglob = qc * 4 + tt
                ptq2 = ps_tr.tile([128, 256], F32, tag="ps_tr",
                                  name="ptq2")
                for mm in range(2):
                    nc.tensor.transpose(
                        ptq2[:, mm * 128:(mm + 1) * 128],
                        oq[:, mm * 128:(mm + 1) * 128], ident[:])
                nc.vector.tensor_copy(
                    oqT[b][:].rearrange("p (a t) -> p a t", a=2)
                    [:, :, tglob * 128:(tglob + 1) * 128],
                    ptq2[:].rearrange("p (a t) -> p a t", a=2))

        def oproj_chunk(b, tch):
            t0 = b * S
            tc0 = tch * 512
            for mo in range(16):
                po = ps_po.tile([128, 512], F32, tag="po")
                for i in range(2):
                    nc.tensor.matmul(
                        po[:],
                        woT[:, i * HID + mo * 128:
                            i * HID + (mo + 1) * 128],
                        oqT[b][:, i * S + tc0: i * S + tc0 + 512],
                        start=(i == 0), stop=(i == 1))
                posb = sb_io.tile([128, 512], BF16, tag="posb",
                                  name="posb")
                if b == 0 and mo % 2 == 0:
                    nc.scalar.copy(posb[:], po[:])
                else:
                    nc.vector.tensor_copy(posb[:], po[:])
                nc.sync.dma_start(
                    out_d[mo * 128:(mo + 1) * 128,
                          t0 + tc0:t0 + tc0 + 512],
                    posb[:])

        # ---- batch 0 projections; x_prep(1)/wo interleaved ----
        wo_prep()
        for cc in range(NCH):
            proj_chunk(0, cc)
        # ---- b0 attention interleaved with b1 projections; b0's o_proj
        # is deferred into the tail (PE filler while b1 attention keeps
        # the ACT engine busy); b1 attention runs longest-qc first so the
        # final serial drain is the shortest qc ----
        for qc in range(4):
            attention_qc(0, qc)
            oproj_chunk(0, qc)
            proj_chunk(1, qc)
        for qc in (3, 2, 1, 0):
            attention_qc(1, qc)
            oproj_chunk(1, qc)

    nc.compile()
    return nc


def _np_quant(x):
    """Host fp4 fake-quant, bit-exact to the device implementation."""
    sh = x.shape
    xb = x.reshape(sh[:-1] + (sh[-1] // 16, 16)).astype(np.float32)
    amax = np.max(np.abs(xb), axis=-1, keepdims=True).astype(np.float32)
    amax_c = np.maximum(amax, np.float32(1e-30))
    rcp = (np.float32(1.0) / amax_c).astype(np.float32)
    rs6 = (rcp * np.float32(6.0)).astype(np.float32)
    scale = (amax * np.float32(1.0 / 6.0)).astype(np.float32)
    y = (xb * rs6).astype(np.float32)
    yi = y.view(np.int32)
    rem = yi & 0x3FFFFF
    inc = (rem > 0x200000).astype(np.int32) << 22
    h = ((yi & np.int32(-4194304)) + inc).view(np.float32)
    M32 = np.float32(MAGIC)
    low = ((y + M32).astype(np.float32) - M32).astype(np.float32)
    q = np.where(np.abs(y) > np.float32(2.0), h, low)
    return (q * scale).astype(np.float32).reshape(sh)


_HOST_CACHE = {}


def _host_tables():
    if _HOST_CACHE:
        return _HOST_CACHE
    D = HD
    inv = (1.0 / (10000.0 ** (np.arange(0, D, 2, dtype=np.float32)
                              / np.float32(D)))).astype(np.float32)
    fr = (np.arange(S, dtype=np.float32)[:, None] * inv[None, :]).astype(
        np.float32)
    cos = np.concatenate([np.cos(fr), np.cos(fr)], -1).astype(np.float32)
    sin = np.concatenate([np.sin(fr), np.sin(fr)], -1).astype(np.float32)
    cosT = np.zeros((128, T), np.float32)
    sinTs = np.zeros((128, T), np.float32)
    sgn = np.where(np.arange(D) < D // 2, np.float32(-1.0), np.float32(1.0))
    for bb in range(B):
        cosT[:, bb * S:(bb + 1) * S] = np.tile(cos.T, (2, 1))
        sinTs[:, bb * S:(bb + 1) * S] = np.tile((sin * sgn[None, :]).T,
                                                (2, 1))
    # mask table [128, 256], sT layout: col j (global q = qs + j), row k:
    # masked (NEG) iff (j - 128) < k.  cols 0-127: fully masked (used for
    # padded diagonal blocks); cols 128-255: the standard triangle.
    masks = np.zeros((128, 256), np.float32)
    for kk in range(128):
        masks[kk, :128 + kk] = NEG
    _HOST_CACHE.update(cosT=cosT, sinTs=sinTs, masks=masks)
    return _HOST_CACHE


_NC_CACHE = []


def make_in_maps(hidden_states, Wq, Wk, Wv, Wo):
    tabs = _host_tables()
    xf = hidden_states.reshape(T, HID)
    xq16 = np.ascontiguousarray(
        _np_quant(np.asarray(xf, np.float32)).T.reshape(16, 128, T)
        .astype(np.float16))
    wq_q = _np_quant(np.asarray(Wq, np.float32))
    wk_q = _np_quant(np.asarray(Wk, np.float32))
    wv_q = _np_quant(np.asarray(Wv, np.float32))
    wo_q = _np_quant(np.asarray(Wo, np.float32))
    import ml_dtypes
    bf16 = ml_dtypes.bfloat16
    in_maps = []
    for c in range(NCORES):
        sl = slice(c * OD, (c + 1) * OD)
        wqT = np.ascontiguousarray(
            wq_q[sl, :].T.reshape(16, 128, OD).astype(np.float16))
        wkT = np.ascontiguousarray(
            wk_q[sl, :].T.reshape(16, 128, OD).astype(np.float16))
        wvT = np.ascontiguousarray(
            wv_q[sl, :].T.reshape(16, 128, OD).astype(np.float16))
        woTc = np.ascontiguousarray(
            wo_q[:, sl].T.reshape(2, 128, HID).astype(bf16))
        in_maps.append(dict(
            xqT=xq16,
            wqT=wqT, wkT=wkT, wvT=wvT, woT=woTc,
            cosT=tabs['cosT'], sinTs=tabs['sinTs'], masks=tabs['masks'],
        ))
    return in_maps


def kernel(hidden_states, Wq, Wk, Wv, Wo):
    in_maps = make_in_maps(hidden_states, Wq, Wk, Wv, Wo)
    if not _NC_CACHE:
        _NC_CACHE.append(build())
    nc = _NC_CACHE[0]
    res = bass_utils.run_bass_kernel_spmd(nc, in_maps,
                                          core_ids=list(range(NCORES)))
    total = np.zeros((HID, T), np.float32)
    for r in res.results:
        total += np.asarray(r["partialT"], dtype=np.float32)
    return np.ascontiguousarray(total.T.reshape(B, S, HID))


if __name__ == "__main__":
    d = np.load('/root/problem/inputs.npz')
    out = kernel(d['hidden_states'], d['Wq'], d['Wk'], d['Wv'], d['Wo'])
    ref = np.load('/root/problem/ref_out.npy')
    rel2 = np.linalg.norm(out - ref) / np.linalg.norm(ref)
    print(f"relL2={rel2:.3e} absmax={np.abs(out - ref).max():.3e}")
